# revision 1
# baseline (speedup 1.0000x reference)
"""Trainium2 Bass kernel for nn_AttentionBlock (complex attention block).

Shapes: B=2, C=128, H=W=64 -> s=4096 tokens, NUM_HEADS=4, dh=32.
Sharding: 8 cores = (batch b, seq-quarter qi). Each core computes the full
attention output for 1024 query tokens of one batch element (all 4 heads),
so there are no collectives: the final channel-mixing projection is local.

Math restructuring (host-side, exact):
  - LN affine (w, b) folded into the QKV in-projection weights/bias.
  - out-projection and 1x1 conv composed into one complex matrix
    M = Wc @ Wout and bias bM = Wc @ out_b + conv_b.
Device pipeline per core:
  P1: stream x[b] in 8 blocks of 512 tokens: LN stats via 1/128-ones matmul
      (partition reduction + free partition-broadcast), var from centered
      squares, inv-std via exp(-0.5*ln(var+eps)) on ACT (single table set),
      yn = (x-mu)*inv, then K/V projection matmuls (fp32r).
      Same pipeline on the core's 1024-token x-quarter for Q.
  P2: PE-transpose V to token-major (bf16) for the PV matmul.
  P3: per (s-block 512, head): scores^T = (k_cat)^T-style matmul (K=64,
      fp32r), exp on ACT (PSUM->SBUF, bf16, scale=1/sqrt(dh) folded in),
      PV matmul accumulates [sum(exp*v_r); sum(exp*v_i)] while a concurrent
      ones-matmul builds the broadcast softmax denominator; normalize with
      r = exp(-ln(den)); project heads with M (PSUM-accumulated), add
      residual + bias, DMA out.
"""

import os
import sys
from contextlib import ExitStack

import numpy as np

sys.path.insert(0, "/opt/trn_rl_repo")

B, C, S, SQ = 2, 128, 4096, 1024
NH, DH = 4, 32
EPS = 1e-5
SCALE = 1.0 / np.sqrt(np.float32(DH))
NB = S // 512   # 8 full-seq blocks
NQB = SQ // 512  # 2 query blocks
NT = S // 128   # 32 key/value token chunks
F32 = None  # set after mybir import
LAST_RESULTS = None


def build_program():
    import concourse.bass as bass
    import concourse.mybir as mybir
    import concourse.tile as tile
    from concourse.masks import make_identity

    f32 = mybir.dt.float32
    f32r = mybir.dt.float32r
    bf16 = mybir.dt.bfloat16
    AF = mybir.ActivationFunctionType
    OP = mybir.AluOpType

    def split_multi_waits(nc):
        """walrus on this image encodes at most ONE sync wait per
        instruction; split extras into same-engine NOPs placed before."""
        def fix_block(blk):
            new_insts = []
            for inst in blk.instructions:
                try:
                    subs = inst.blocks
                except AttributeError:
                    subs = None
                if subs:
                    for sub in subs:
                        fix_block(sub)
                si = inst.sync_info
                waits = list(si.on_wait) if si is not None and si.on_wait else []
                if len(waits) > 1:
                    for j, w in enumerate(waits[:-1]):
                        nop = mybir.InstNoOp(name=f"{inst.name}-ws{j}")
                        nop.engine = inst.engine
                        nop.sync_info = mybir.SyncInfo(on_wait=[w],
                                                       on_update=[])
                        new_insts.append(nop)
                    inst.sync_info = mybir.SyncInfo(
                        on_wait=[waits[-1]], on_update=list(si.on_update))
                new_insts.append(inst)
            blk.instructions = new_insts
        for blk in nc.m.functions[0].blocks:
            fix_block(blk)

    nc = bass.Bass()

    xr_t = nc.declare_dram_parameter("xr", [C, S], f32, isOutput=False)
    xi_t = nc.declare_dram_parameter("xi", [C, S], f32, isOutput=False)
    xqr_t = nc.declare_dram_parameter("xqr", [C, SQ], f32, isOutput=False)
    xqi_t = nc.declare_dram_parameter("xqi", [C, SQ], f32, isOutput=False)
    wa_t = nc.declare_dram_parameter("wa", [C, 768], bf16, isOutput=False)
    wb_t = nc.declare_dram_parameter("wb", [C, 768], bf16, isOutput=False)
    qkvb_t = nc.declare_dram_parameter("qkvb", [C, 6], f32, isOutput=False)
    lp_t = nc.declare_dram_parameter("lp", [C, 512], bf16, isOutput=False)
    pb_t = nc.declare_dram_parameter("pb", [1, 256], bf16, isOutput=False)
    out_t = nc.declare_dram_parameter("out", [2, C, SQ], f32, isOutput=True)

    with tile.TileContext(nc) as tc, ExitStack() as ctx:
        # ---- persistent pools ----
        const_pool = ctx.enter_context(tc.tile_pool(name="const", bufs=1))
        big_pool = ctx.enter_context(tc.tile_pool(name="big", bufs=1))

        ident = const_pool.tile([128, 128], bf16, tag="ident", name="ident")
        make_identity(nc, ident[:])
        ones_bc = const_pool.tile([128, 128], bf16, tag="ones_bc", name="ones_bc")
        nc.gpsimd.memset(ones_bc[:], 1.0 / 128.0)
        ones_pv = const_pool.tile([128, 64], bf16, tag="ones_pv", name="ones_pv")
        nc.gpsimd.memset(ones_pv[:], 1.0)
        ones_row = const_pool.tile([1, 512], bf16, tag="ones_row", name="ones_row")
        nc.gpsimd.memset(ones_row[:], 1.0)
        eps_c = const_pool.tile([128, 1], f32, tag="eps_c", name="eps_c")
        nc.gpsimd.memset(eps_c[:], EPS)
        # Pre-sync ACT with gpsimd consts (and trigger the exp/ln table
        # load early) so later activations carry a single sync wait.
        act_warm = const_pool.tile([128, 1], f32, tag="act_warm",
                                   name="act_warm")
        nc.scalar.activation(act_warm[:], eps_c[:], AF.Exp)

        wa = const_pool.tile([C, 768], bf16, tag="wa", name="wa")
        wb = const_pool.tile([C, 768], bf16, tag="wb", name="wb")
        qkvb = const_pool.tile([C, 6], f32, tag="qkvb", name="qkvb")
        lp = const_pool.tile([C, 512], bf16, tag="lp", name="lp")
        pb = const_pool.tile([1, 256], bf16, tag="pb", name="pb")
        nc.sync.dma_start(out=wa[:], in_=wa_t[:])
        nc.sync.dma_start(out=wb[:], in_=wb_t[:])
        nc.sync.dma_start(out=qkvb[:], in_=qkvb_t[:])
        nc.sync.dma_start(out=lp[:], in_=lp_t[:])
        nc.sync.dma_start(out=pb[:], in_=pb_t[:])
        # Pre-sync DVE with the qkvb DMA lane so the qkv copy-out TS ops
        # carry a single sync wait (walrus wait-slot limit).
        dve_warm = const_pool.tile([128, 1], f32, tag="dve_warm",
                                   name="dve_warm")
        nc.vector.tensor_copy(dve_warm[:], qkvb[:, 0:1])

        # persistent activation storage
        ksb = [big_pool.tile([128, S], bf16, tag=f"ksb{p}", name=f"ksb{p}") for p in range(2)]
        qsb = [big_pool.tile([128, SQ], bf16, tag=f"qsb{p}", name=f"qsb{p}") for p in range(2)]
        vT = [big_pool.tile([128, S], bf16, tag=f"vT{p}", name=f"vT{p}") for p in range(2)]
        resid = [big_pool.tile([128, 512], f32, tag=f"res{i}", name=f"res{i}") for i in range(4)]
        # resid order: [r blk0, r blk1, i blk0, i blk1]

        # ---------------- P1: LN + QKV projection ----------------
        def ln_block(xin_pool, stat_pool, tmp_pool, src_r, src_i, dma=True):
            """Returns (yn_r, yn_i) SBUF tiles [128,512] for one 512-token block."""
            if dma:
                xr_b = xin_pool.tile([128, 512], f32, tag="xr_b", name="xr_b")
                xi_b = xin_pool.tile([128, 512], f32, tag="xi_b", name="xi_b")
                nc.sync.dma_start(out=xr_b[:], in_=src_r)
                nc.sync.dma_start(out=xi_b[:], in_=src_i)
                xr_ap, xi_ap = xr_b[:], xi_b[:]
            else:
                xr_ap, xi_ap = src_r, src_i
            xb_r = tmp_pool.tile([128, 512], bf16, tag="xb_r", name="xb_r")
            xb_i = tmp_pool.tile([128, 512], bf16, tag="xb_i", name="xb_i")
            nc.vector.tensor_copy(xb_r[:], xr_ap)
            nc.vector.tensor_copy(xb_i[:], xi_ap)
            mu_r = stat_pool.tile([128, 512], f32, tag="mu_r", name="mu_r")
            mu_i = stat_pool.tile([128, 512], f32, tag="mu_i", name="mu_i")
            nc.tensor.matmul(mu_r[:], ones_bc[:], xb_r[:],
                             start=True, stop=True)
            nc.tensor.matmul(mu_i[:], ones_bc[:], xb_i[:],
                             start=True, stop=True)
            d_r = tmp_pool.tile([128, 512], f32, tag="d_r", name="d_r")
            d_i = tmp_pool.tile([128, 512], f32, tag="d_i", name="d_i")
            nc.vector.tensor_tensor(d_r[:], xr_ap, mu_r[:], OP.subtract)
            nc.vector.tensor_tensor(d_i[:], xi_ap, mu_i[:], OP.subtract)
            sq_r = tmp_pool.tile([128, 512], bf16, tag="sq_r", name="sq_r")
            sq_i = tmp_pool.tile([128, 512], bf16, tag="sq_i", name="sq_i")
            nc.vector.tensor_tensor(sq_r[:], d_r[:], d_r[:], OP.mult)
            nc.vector.tensor_tensor(sq_i[:], d_i[:], d_i[:], OP.mult)
            var = stat_pool.tile([128, 512], f32, tag="var", name="var")
            nc.tensor.matmul(var[:], ones_bc[:], sq_r[:],
                             start=True, stop=False)
            nc.tensor.matmul(var[:], ones_bc[:], sq_i[:],
                             start=False, stop=True)
            lntmp = tmp_pool.tile([128, 512], f32, tag="lntmp", name="lntmp")
            nc.scalar.activation(lntmp[:], var[:], AF.Ln, bias=eps_c[:])
            inv = tmp_pool.tile([128, 512], f32, tag="inv", name="inv")
            nc.scalar.activation(inv[:], lntmp[:], AF.Exp, scale=-0.5)
            yn_r = tmp_pool.tile([128, 512], bf16, tag="yn_r", name="yn_r")
            yn_i = tmp_pool.tile([128, 512], bf16, tag="yn_i", name="yn_i")
            nc.vector.tensor_tensor(yn_r[:], d_r[:], inv[:], OP.mult)
            nc.vector.tensor_tensor(yn_i[:], d_i[:], inv[:], OP.mult)
            return yn_r, yn_i

        def qkv_tile(qkv_pool, t, yn_r, yn_i, dest_slice):
            ps = qkv_pool.tile([128, 512], f32, tag="qkv_ps", name="qkv_ps")
            nc.tensor.matmul(ps[:], wa[:, t * 128:(t + 1) * 128],
                             yn_r[:], start=True, stop=False)
            nc.tensor.matmul(ps[:], wb[:, t * 128:(t + 1) * 128],
                             yn_i[:], start=False, stop=True)
            nc.vector.tensor_scalar_add(dest_slice, ps[:], qkvb[:, t:t + 1])

        with ExitStack() as pvs:
            vsb = [pvs.enter_context(tc.tile_pool(name=f"vsb{p}", bufs=1))
                   .tile([128, S], bf16, tag=f"vsb{p}", name=f"vsb{p}") for p in range(2)]

            with ExitStack() as p1:
                xin_pool = p1.enter_context(tc.tile_pool(name="xin", bufs=8))
                tmp_pool = p1.enter_context(tc.tile_pool(name="tmp", bufs=2))
                stat_pool = p1.enter_context(
                    tc.tile_pool(name="stat", bufs=1, space="PSUM"))
                qkv_pool = p1.enter_context(
                    tc.tile_pool(name="qkvps", bufs=4, space="PSUM"))

                for blk in range(NB):
                    sl = slice(blk * 512, (blk + 1) * 512)
                    yn_r, yn_i = ln_block(xin_pool, stat_pool, tmp_pool,
                                          xr_t[:, sl], xi_t[:, sl])
                    qkv_tile(qkv_pool, 0, yn_r, yn_i, ksb[0][:, sl])
                    qkv_tile(qkv_pool, 1, yn_r, yn_i, ksb[1][:, sl])
                    qkv_tile(qkv_pool, 2, yn_r, yn_i, vsb[0][:, sl])
                    qkv_tile(qkv_pool, 3, yn_r, yn_i, vsb[1][:, sl])
                for qb in range(NQB):
                    sl = slice(qb * 512, (qb + 1) * 512)
                    rr, ri = resid[qb], resid[2 + qb]
                    nc.sync.dma_start(out=rr[:], in_=xqr_t[:, sl])
                    nc.sync.dma_start(out=ri[:], in_=xqi_t[:, sl])
                    yn_r, yn_i = ln_block(xin_pool, stat_pool, tmp_pool,
                                          rr[:], ri[:], dma=False)
                    qkv_tile(qkv_pool, 4, yn_r, yn_i, qsb[0][:, sl])
                    qkv_tile(qkv_pool, 5, yn_r, yn_i, qsb[1][:, sl])

            # ---------------- P2: transpose V to token-major ----------------
            with ExitStack() as p2:
                tp_pool = p2.enter_context(
                    tc.tile_pool(name="tp", bufs=6, space="PSUM"))
                for p in range(2):
                    for tt in range(NT):
                        tsl = slice(tt * 128, (tt + 1) * 128)
                        ps = tp_pool.tile([128, 128], bf16, tag="tp", name="tp")
                        nc.tensor.transpose(ps[:], vsb[p][:, tsl], ident[:])
                        nc.vector.tensor_copy(vT[p][:, tsl], ps[:])

        # ---------------- P3: attention + projection ----------------
        GR = 2  # score chunks per exp granule (2 PSUM banks)
        with ExitStack() as p3:
            sc_pool = p3.enter_context(
                tc.tile_pool(name="scps", bufs=3, space="PSUM"))
            pv_pool = p3.enter_context(
                tc.tile_pool(name="pvps", bufs=1, space="PSUM"))
            exp_pool = p3.enter_context(tc.tile_pool(name="exp", bufs=2))
            sm_pool = p3.enter_context(tc.tile_pool(name="sm", bufs=3))
            out_pool = p3.enter_context(tc.tile_pool(name="outp", bufs=4))

            for blk in range(NQB):
                bsl = slice(blk * 512, (blk + 1) * 512)
                attn = [sm_pool.tile([128, 512], bf16, tag=f"attn{p}", name=f"attn{p}")
                        for p in range(2)]
                for h in range(NH):
                    pair, off = h // 2, 64 * (h % 2)
                    q_ap = qsb[pair][off:off + 64, bsl]
                    exp_sb = exp_pool.tile([128, NT * 512], bf16, tag="exp", name="exp")
                    pv = pv_pool.tile([64, 512], f32, tag="pv", name="pv")
                    den = pv_pool.tile([128, 512], f32, tag="den", name="den")
                    tt0 = 0
                    while tt0 < NT:
                        g = min(GR, NT - tt0)
                        sc = sc_pool.tile([128, GR * 512], f32, tag="sc", name="sc")
                        for j in range(g):
                            tt = tt0 + j
                            nc.tensor.matmul(
                                sc[:, j * 512:(j + 1) * 512],
                                ksb[pair][off:off + 64,
                                              tt * 128:(tt + 1) * 128],
                                q_ap, start=True, stop=True)
                        nc.scalar.activation(
                            exp_sb[:, tt0 * 512:(tt0 + g) * 512],
                            sc[:, 0:g * 512], AF.Exp, scale=float(SCALE))
                        for j in range(g):
                            tt = tt0 + j
                            esl = exp_sb[:, tt * 512:(tt + 1) * 512]
                            nc.tensor.matmul(
                                pv[:],
                                vT[pair][:, tt * 128 + off:tt * 128 + off + 64],
                                esl, start=(tt == 0), stop=(tt == NT - 1))
                            nc.tensor.matmul(
                                den[64:128, :], ones_pv[:], esl,
                                start=(tt == 0), stop=(tt == NT - 1),
                                tile_position=(0, 64))
                        tt0 += g
                    # tiny DVE pre-read of pv so the normalize TT below only
                    # waits on ACT (single-wait-slot limit)
                    pvsync = sm_pool.tile([1, 1], f32, tag="pvsync",
                                          name="pvsync")
                    nc.vector.tensor_copy(pvsync[:], pv[0:1, 0:1])
                    rtmp = sm_pool.tile([64, 512], f32, tag="rtmp", name="rtmp")
                    nc.scalar.activation(rtmp[:], den[64:128, :], AF.Ln)
                    r_bc = sm_pool.tile([64, 512], f32, tag="r_bc", name="r_bc")
                    nc.scalar.activation(r_bc[:], rtmp[:], AF.Exp, scale=-1.0)
                    nc.vector.tensor_tensor(attn[pair][off:off + 64, :],
                                            pv[:], r_bc[:], OP.mult)
                # head-mixing projection + residual + bias
                for comp in range(2):  # 0=real, 1=imag
                    ps = pv_pool.tile([128, 512], f32,
                                      tag=("pv" if comp == 0 else "den"),
                                      name="proj")
                    nc.tensor.matmul(ps[:], lp[:, comp * 128:(comp + 1) * 128],
                                     attn[0][:], start=True, stop=False)
                    nc.tensor.matmul(ps[:],
                                     lp[:, 256 + comp * 128:384 + comp * 128],
                                     attn[1][:], start=False, stop=False)
                    nc.tensor.matmul(ps[:],
                                     pb[0:1, comp * 128:(comp + 1) * 128],
                                     ones_row[:], start=False, stop=True)
                    o_sb = out_pool.tile([128, 512], f32, tag="o_sb", name="o_sb")
                    nc.vector.tensor_tensor(o_sb[:], ps[:],
                                            resid[2 * comp + blk][:], OP.add)
                    nc.sync.dma_start(out=out_t[comp, :, bsl], in_=o_sb[:])
    split_multi_waits(nc)
    return nc


def pack_inputs(inputs):
    """Host-side exact restructuring; returns per-core input maps."""
    f = lambda k: np.asarray(inputs[k], np.float32)
    xr = f("x_real").reshape(B, C, S)
    xi = f("x_imag").reshape(B, C, S)
    Win = (f("in_w_r") + 1j * f("in_w_i")).astype(np.complex64)
    lnw = (f("ln_w_r") + 1j * f("ln_w_i")).astype(np.complex64)
    lnb = (f("ln_b_r") + 1j * f("ln_b_i")).astype(np.complex64)
    inb = (f("in_b_r") + 1j * f("in_b_i")).astype(np.complex64)
    Wp = Win * lnw[None, :]
    biasq = inb + Win @ lnb
    Wout = (f("out_w_r") + 1j * f("out_w_i")).astype(np.complex64)
    Wc = (f("conv_w_r") + 1j * f("conv_w_i")).astype(np.complex64)
    outb = (f("out_b_r") + 1j * f("out_b_i")).astype(np.complex64)
    convb = (f("conv_b_r") + 1j * f("conv_b_i")).astype(np.complex64)
    M = Wc @ Wout
    bM = Wc @ outb + convb

    def pack_pair(Wsec, bsec, h0):
        W0 = Wsec[32 * h0:32 * h0 + 32]
        W1 = Wsec[32 * (h0 + 1):32 * (h0 + 1) + 32]
        b0 = bsec[32 * h0:32 * h0 + 32]
        b1 = bsec[32 * (h0 + 1):32 * (h0 + 1) + 32]
        RA = np.concatenate([W0.real, W0.imag, W1.real, W1.imag], 0)
        RB = np.concatenate([-W0.imag, W0.real, -W1.imag, W1.real], 0)
        bcol = np.concatenate([b0.real, b0.imag, b1.real, b1.imag], 0)
        return RA.T.copy(), RB.T.copy(), bcol

    qW, kW, vW = Wp[0:C], Wp[C:2 * C], Wp[2 * C:3 * C]
    qb_, kb_, vb_ = biasq[0:C], biasq[C:2 * C], biasq[2 * C:3 * C]
    tiles = [pack_pair(kW, kb_, 0), pack_pair(kW, kb_, 2),
             pack_pair(vW, vb_, 0), pack_pair(vW, vb_, 2),
             pack_pair(qW, qb_, 0), pack_pair(qW, qb_, 2)]
    import ml_dtypes
    bf = ml_dtypes.bfloat16
    wa = np.ascontiguousarray(np.concatenate([t[0] for t in tiles], 1)).astype(bf)
    wb = np.ascontiguousarray(np.concatenate([t[1] for t in tiles], 1)).astype(bf)
    qkvb = np.ascontiguousarray(np.stack([t[2] for t in tiles], 1), np.float32)

    def pack_proj(h0):
        M0 = M[:, 32 * h0:32 * h0 + 32]
        M1 = M[:, 32 * (h0 + 1):32 * (h0 + 1) + 32]
        Lr = np.concatenate([M0.real.T, -M0.imag.T, M1.real.T, -M1.imag.T], 0)
        Li = np.concatenate([M0.imag.T, M0.real.T, M1.imag.T, M1.real.T], 0)
        return Lr, Li
    L01r, L01i = pack_proj(0)
    L23r, L23i = pack_proj(2)
    lp = np.ascontiguousarray(np.concatenate([L01r, L01i, L23r, L23i], 1)).astype(bf)
    pb = np.ascontiguousarray(
        np.concatenate([bM.real, bM.imag])[None, :]).astype(bf)

    in_maps = []
    for core in range(8):
        b, qi = core // 4, core % 4
        qsl = slice(qi * SQ, (qi + 1) * SQ)
        in_maps.append({
            "xr": np.ascontiguousarray(xr[b]),
            "xi": np.ascontiguousarray(xi[b]),
            "xqr": np.ascontiguousarray(xr[b][:, qsl]),
            "xqi": np.ascontiguousarray(xi[b][:, qsl]),
            "wa": wa, "wb": wb, "qkvb": qkvb, "lp": lp, "pb": pb,
        })
    return in_maps


_CACHED = {}


def _ensure_ntff_hook():
    """Register the axon NTFF profiling hook (absent from this image's
    antenv) so run_bass_kernel_spmd(trace=True) can capture HW timing."""
    try:
        import antenv.axon_hooks  # noqa: F401
        return
    except ImportError:
        pass
    import types

    try:
        from trn_agent_boot import trn_boot
        hook = trn_boot._ntff_profile_via_ctypes("/opt/axon/libaxon_pjrt.so")
    except Exception:
        return
    import antenv

    mod = types.ModuleType("antenv.axon_hooks")
    mod.get_axon_ntff_profile_hook = lambda: hook
    mod.set_axon_ntff_profile_hook = lambda h: None
    sys.modules["antenv.axon_hooks"] = mod
    antenv.axon_hooks = mod


def kernel(trace=False, **inputs):
    global LAST_RESULTS
    from concourse.bass_utils import run_bass_kernel_spmd

    if trace:
        _ensure_ntff_hook()

    if "nc" not in _CACHED:
        _CACHED["nc"] = build_program()
    nc = _CACHED["nc"]
    in_maps = pack_inputs(inputs)
    res = run_bass_kernel_spmd(nc, in_maps, core_ids=list(range(8)),
                               trace=trace)
    LAST_RESULTS = res
    out = np.zeros((2, B, C, S), np.float32)
    for core in range(8):
        b, qi = core // 4, core % 4
        out[:, b, :, qi * SQ:(qi + 1) * SQ] = res.results[core]["out"]
    return out.reshape(2, B, C, 64, 64)



# revision 2
# speedup vs baseline: 1.0888x; 1.0888x over previous
"""Trainium2 Bass kernel for nn_AttentionBlock (complex attention block).

Shapes: B=2, C=128, H=W=64 -> s=4096 tokens, NUM_HEADS=4, dh=32.
Sharding: 8 cores = (batch b, seq-quarter qi). Each core computes the full
attention output for 1024 query tokens of one batch element (all 4 heads),
so there are no collectives: the final channel-mixing projection is local.

Math restructuring (host-side, exact):
  - LN affine (w, b) folded into the QKV in-projection weights/bias.
  - K-bias dropped entirely (softmax is invariant to per-query shifts).
  - V-bias folded into the output-projection bias (attn rows sum to 1).
  - out-projection and 1x1 conv composed into one complex matrix
    M = Wc @ Wout; bias bM = Wc @ out_b + conv_b + M @ v_bias.
  - Key order is permuted per-core so each core's own query quarter is
    always blocks 6-7 (softmax/PV are permutation-invariant over keys),
    letting one 8-block pass produce K, V, Q and the residual.
Device pipeline per core:
  P1: stream x in 8 blocks of 512 tokens: LN stats via 1/128-ones matmul,
      bf16 elementwise chain on DVE, inv-std via exp(-0.5*ln(var+eps)),
      K projection -> ksb [feat, tok]; V projected directly token-major
      (yn_chunk^T @ Wv per 128-token chunk -> no PE transposes); on the
      last two blocks also Q projection + residual capture.
  P2: per (query-block 512, head-pair): per key chunk of 128 tokens:
      two row-tiled concurrent score matmuls (K=64 each at PE rows 0-63 /
      64-127), one exp over [128,1024] on ACT, then col-tiled PV and
      denominator matmuls accumulating into shared [128,512] PSUM banks;
      normalize with one Ln + one Exp + one multiply per pair; project
      heads with M, add residual + bias, DMA out.
"""

import os
import sys
from contextlib import ExitStack

import numpy as np

sys.path.insert(0, "/opt/trn_rl_repo")

B, C, S, SQ = 2, 128, 4096, 1024
NH, DH = 4, 32
EPS = 1e-5
SCALE = 1.0 / np.sqrt(np.float32(DH))
NB = S // 512    # 8 blocks of 512 tokens
NT = S // 128    # 32 key/value token chunks
LAST_RESULTS = None


def build_program():
    import concourse.bass as bass
    import concourse.mybir as mybir
    import concourse.tile as tile

    f32 = mybir.dt.float32
    bf16 = mybir.dt.bfloat16
    AF = mybir.ActivationFunctionType
    OP = mybir.AluOpType

    def split_multi_waits(nc):
        """walrus on this image encodes at most ONE sync wait per
        instruction; split extras into same-engine NOPs placed before."""
        def fix_block(blk):
            new_insts = []
            for inst in blk.instructions:
                try:
                    subs = inst.blocks
                except AttributeError:
                    subs = None
                if subs:
                    for sub in subs:
                        fix_block(sub)
                si = inst.sync_info
                waits = list(si.on_wait) if si is not None and si.on_wait else []
                if len(waits) > 1:
                    for j, w in enumerate(waits[:-1]):
                        nop = mybir.InstNoOp(name=f"{inst.name}-ws{j}")
                        nop.engine = inst.engine
                        nop.sync_info = mybir.SyncInfo(on_wait=[w],
                                                       on_update=[])
                        new_insts.append(nop)
                    inst.sync_info = mybir.SyncInfo(
                        on_wait=[waits[-1]], on_update=list(si.on_update))
                new_insts.append(inst)
            blk.instructions = new_insts
        for blk in nc.m.functions[0].blocks:
            fix_block(blk)

    nc = bass.Bass()

    xr_t = nc.declare_dram_parameter("xr", [C, S], f32, isOutput=False)
    xi_t = nc.declare_dram_parameter("xi", [C, S], f32, isOutput=False)
    # K/Q projection weights: [k_p0 | k_p1 | q_p0 | q_p1], 128 cols each
    wa_t = nc.declare_dram_parameter("wa", [C, 512], bf16, isOutput=False)
    wb_t = nc.declare_dram_parameter("wb", [C, 512], bf16, isOutput=False)
    # V projection (token-major output): [p0h0|p0h1|p1h0|p1h1], 64 cols each
    wva_t = nc.declare_dram_parameter("wva", [C, 256], bf16, isOutput=False)
    wvb_t = nc.declare_dram_parameter("wvb", [C, 256], bf16, isOutput=False)
    qb2_t = nc.declare_dram_parameter("qb2", [C, 2], f32, isOutput=False)
    lp_t = nc.declare_dram_parameter("lp", [C, 512], bf16, isOutput=False)
    pb_t = nc.declare_dram_parameter("pb", [1, 256], bf16, isOutput=False)
    out_t = nc.declare_dram_parameter("out", [2, C, SQ], f32, isOutput=True)

    with tile.TileContext(nc) as tc, ExitStack() as ctx:
        const_pool = ctx.enter_context(tc.tile_pool(name="const", bufs=1))
        big_pool = ctx.enter_context(tc.tile_pool(name="big", bufs=1))

        ones_bc = const_pool.tile([128, 128], bf16, tag="ones_bc", name="ones_bc")
        nc.gpsimd.memset(ones_bc[:], 1.0 / 128.0)
        ones_pv = const_pool.tile([128, 64], bf16, tag="ones_pv", name="ones_pv")
        nc.gpsimd.memset(ones_pv[:], 1.0)
        ones_row = const_pool.tile([1, 512], bf16, tag="ones_row", name="ones_row")
        nc.gpsimd.memset(ones_row[:], 1.0)
        eps_c = const_pool.tile([128, 1], f32, tag="eps_c", name="eps_c")
        nc.gpsimd.memset(eps_c[:], EPS)
        # Pre-sync ACT with gpsimd consts (and trigger the exp/ln table
        # load early) so later activations carry a single sync wait.
        act_warm = const_pool.tile([128, 1], f32, tag="act_warm",
                                   name="act_warm")
        nc.scalar.activation(act_warm[:], eps_c[:], AF.Exp)

        wa = const_pool.tile([C, 512], bf16, tag="wa", name="wa")
        wb = const_pool.tile([C, 512], bf16, tag="wb", name="wb")
        wva = const_pool.tile([C, 256], bf16, tag="wva", name="wva")
        wvb = const_pool.tile([C, 256], bf16, tag="wvb", name="wvb")
        qb2 = const_pool.tile([C, 2], f32, tag="qb2", name="qb2")
        lp = const_pool.tile([C, 512], bf16, tag="lp", name="lp")
        pb = const_pool.tile([1, 256], bf16, tag="pb", name="pb")
        nc.sync.dma_start(out=wa[:], in_=wa_t[:])
        nc.sync.dma_start(out=wb[:], in_=wb_t[:])
        nc.sync.dma_start(out=wva[:], in_=wva_t[:])
        nc.sync.dma_start(out=wvb[:], in_=wvb_t[:])
        nc.sync.dma_start(out=qb2[:], in_=qb2_t[:])
        nc.sync.dma_start(out=lp[:], in_=lp_t[:])
        nc.sync.dma_start(out=pb[:], in_=pb_t[:])
        # Pre-sync DVE with the weight DMA lane so later DVE ops carry a
        # single sync wait (walrus wait-slot limit).
        dve_warm = const_pool.tile([128, 1], f32, tag="dve_warm",
                                   name="dve_warm")
        nc.vector.tensor_copy(dve_warm[:], qb2[:, 0:1])

        # persistent activation storage
        ksb = [big_pool.tile([128, S], bf16, tag=f"ksb{p}", name=f"ksb{p}")
               for p in range(2)]
        qsb = [big_pool.tile([128, SQ], bf16, tag=f"qsb{p}", name=f"qsb{p}")
               for p in range(2)]
        # vTe: token-major V, per chunk 256 cols [p0h0|p0h1|p1h0|p1h1]
        vTe = big_pool.tile([128, NT * 256], bf16, tag="vTe", name="vTe")
        resid = [big_pool.tile([128, 512], f32, tag=f"res{i}", name=f"res{i}")
                 for i in range(4)]
        # resid order: [r blk0, r blk1, i blk0, i blk1]

        # ---------------- P1: LN + QKV projection ----------------
        with ExitStack() as p1:
            xin_pool = p1.enter_context(tc.tile_pool(name="xin", bufs=6))
            tmp_pool = p1.enter_context(tc.tile_pool(name="tmp", bufs=2))
            mu_pool = p1.enter_context(
                tc.tile_pool(name="mups", bufs=1, space="PSUM"))
            var_pool = p1.enter_context(
                tc.tile_pool(name="varps", bufs=2, space="PSUM"))
            kq_pool = p1.enter_context(
                tc.tile_pool(name="kqps", bufs=2, space="PSUM"))
            vt_pool = p1.enter_context(
                tc.tile_pool(name="vtps", bufs=2, space="PSUM"))

            for blk in range(NB):
                sl = slice(blk * 512, (blk + 1) * 512)
                own = blk >= 6  # own query quarter (host permutes keys)
                qb_i = blk - 6
                if own:
                    xr_b = resid[qb_i]
                    xi_b = resid[2 + qb_i]
                else:
                    xr_b = xin_pool.tile([128, 512], f32, tag="xr_b",
                                         name="xr_b")
                    xi_b = xin_pool.tile([128, 512], f32, tag="xi_b",
                                         name="xi_b")
                nc.sync.dma_start(out=xr_b[:], in_=xr_t[:, sl])
                nc.sync.dma_start(out=xi_b[:], in_=xi_t[:, sl])

                xb_r = tmp_pool.tile([128, 512], bf16, tag="xb_r", name="xb_r")
                xb_i = tmp_pool.tile([128, 512], bf16, tag="xb_i", name="xb_i")
                nc.vector.tensor_copy(xb_r[:], xr_b[:])
                nc.vector.tensor_copy(xb_i[:], xi_b[:])

                mu = mu_pool.tile([128, 1024], f32, tag="mu", name="mu")
                nc.tensor.matmul(mu[:, 0:512], ones_bc[:], xb_r[:],
                                 start=True, stop=True)
                nc.tensor.matmul(mu[:, 512:1024], ones_bc[:], xb_i[:],
                                 start=True, stop=True)
                mu_sb = tmp_pool.tile([128, 1024], bf16, tag="mu_sb",
                                      name="mu_sb")
                nc.scalar.activation(mu_sb[:], mu[:], AF.Copy)

                d_r = tmp_pool.tile([128, 512], bf16, tag="d_r", name="d_r")
                d_i = tmp_pool.tile([128, 512], bf16, tag="d_i", name="d_i")
                nc.vector.tensor_tensor(d_r[:], xb_r[:], mu_sb[:, 0:512],
                                        OP.subtract)
                nc.vector.tensor_tensor(d_i[:], xb_i[:], mu_sb[:, 512:1024],
                                        OP.subtract)
                sq_r = tmp_pool.tile([128, 512], bf16, tag="sq_r", name="sq_r")
                sq_i = tmp_pool.tile([128, 512], bf16, tag="sq_i", name="sq_i")
                nc.vector.tensor_tensor(sq_r[:], d_r[:], d_r[:], OP.mult)
                nc.vector.tensor_tensor(sq_i[:], d_i[:], d_i[:], OP.mult)
                var = var_pool.tile([128, 512], f32, tag="var", name="var")
                nc.tensor.matmul(var[:], ones_bc[:], sq_r[:],
                                 start=True, stop=False)
                nc.tensor.matmul(var[:], ones_bc[:], sq_i[:],
                                 start=False, stop=True)
                lntmp = tmp_pool.tile([128, 512], f32, tag="lntmp",
                                      name="lntmp")
                nc.scalar.activation(lntmp[:], var[:], AF.Ln, bias=eps_c[:])
                inv = tmp_pool.tile([128, 512], bf16, tag="inv", name="inv")
                nc.scalar.activation(inv[:], lntmp[:], AF.Exp, scale=-0.5)
                yn_r = tmp_pool.tile([128, 512], bf16, tag="yn_r", name="yn_r")
                yn_i = tmp_pool.tile([128, 512], bf16, tag="yn_i", name="yn_i")
                nc.vector.tensor_tensor(yn_r[:], d_r[:], inv[:], OP.mult)
                nc.vector.tensor_tensor(yn_i[:], d_i[:], inv[:], OP.mult)

                # K tiles (both pairs), PSUM -> ksb via ACT copy (no bias)
                for p in range(2):
                    ps = kq_pool.tile([128, 512], f32, tag="kq", name="kq")
                    nc.tensor.matmul(ps[:], wa[:, p * 128:(p + 1) * 128],
                                     yn_r[:], start=True, stop=False)
                    nc.tensor.matmul(ps[:], wb[:, p * 128:(p + 1) * 128],
                                     yn_i[:], start=False, stop=True)
                    nc.scalar.activation(ksb[p][:, sl], ps[:], AF.Copy)

                # V token-major: per 128-token chunk, yn_chunk^T @ Wv
                for cch in range(4):
                    tt = blk * 4 + cch
                    csl = slice(cch * 128, (cch + 1) * 128)
                    vt = vt_pool.tile([128, 256], f32, tag="vt", name="vt")
                    nc.tensor.matmul(vt[:], yn_r[:, csl], wva[:],
                                     start=True, stop=False)
                    nc.tensor.matmul(vt[:], yn_i[:, csl], wvb[:],
                                     start=False, stop=True)
                    nc.vector.tensor_copy(
                        vTe[:, tt * 256:(tt + 1) * 256], vt[:])

                if own:
                    qsl = slice(qb_i * 512, (qb_i + 1) * 512)
                    for p in range(2):
                        ps = kq_pool.tile([128, 512], f32, tag="kq",
                                          name="kq")
                        nc.tensor.matmul(ps[:],
                                         wa[:, 256 + p * 128:384 + p * 128],
                                         yn_r[:], start=True, stop=False)
                        nc.tensor.matmul(ps[:],
                                         wb[:, 256 + p * 128:384 + p * 128],
                                         yn_i[:], start=False, stop=True)
                        nc.vector.tensor_scalar_add(qsb[p][:, qsl], ps[:],
                                                    qb2[:, p:p + 1])

        # ---------------- P2: attention + projection ----------------
        with ExitStack() as p3:
            sc_pool = p3.enter_context(
                tc.tile_pool(name="scps", bufs=2, space="PSUM"))
            pv_pool = p3.enter_context(
                tc.tile_pool(name="pvps", bufs=2, space="PSUM"))
            exp_pool = p3.enter_context(tc.tile_pool(name="exp", bufs=3))
            sm_pool = p3.enter_context(tc.tile_pool(name="sm", bufs=2))
            out_pool = p3.enter_context(tc.tile_pool(name="outp", bufs=4))

            for blk in range(2):
                bsl = slice(blk * 512, (blk + 1) * 512)
                attn = [sm_pool.tile([128, 512], bf16, tag=f"attn{p}",
                                     name=f"attn{p}") for p in range(2)]
                for pair in range(2):
                    pv = pv_pool.tile([128, 512], f32, tag="pv", name="pv")
                    den = pv_pool.tile([128, 512], f32, tag="den", name="den")
                    for tt in range(NT):
                        tsl = slice(tt * 128, (tt + 1) * 128)
                        sc = sc_pool.tile([128, 1024], f32, tag="sc",
                                          name="sc")
                        nc.tensor.matmul(sc[:, 0:512],
                                         ksb[pair][0:64, tsl],
                                         qsb[pair][0:64, bsl],
                                         start=True, stop=True)
                        nc.tensor.matmul(sc[:, 512:1024],
                                         ksb[pair][64:128, tsl],
                                         qsb[pair][64:128, bsl],
                                         start=True, stop=True,
                                         tile_position=(64, 0))
                        ex = exp_pool.tile([128, 1024], bf16, tag="ex",
                                           name="ex")
                        nc.scalar.activation(ex[:], sc[:], AF.Exp,
                                             scale=float(SCALE))
                        st, sp = tt == 0, tt == NT - 1
                        vb = tt * 256 + pair * 128
                        nc.tensor.matmul(pv[0:64, :],
                                         vTe[:, vb:vb + 64],
                                         ex[:, 0:512],
                                         start=st, stop=sp,
                                         skip_group_check=True)
                        nc.tensor.matmul(pv[64:128, :],
                                         vTe[:, vb + 64:vb + 128],
                                         ex[:, 512:1024],
                                         start=False, stop=sp,
                                         tile_position=(0, 64),
                                         skip_group_check=True)
                        nc.tensor.matmul(den[0:64, :], ones_pv[:],
                                         ex[:, 0:512],
                                         start=st, stop=sp,
                                         skip_group_check=True)
                        nc.tensor.matmul(den[64:128, :], ones_pv[:],
                                         ex[:, 512:1024],
                                         start=False, stop=sp,
                                         tile_position=(0, 64),
                                         skip_group_check=True)
                    rtmp = sm_pool.tile([128, 512], f32, tag="rtmp",
                                        name="rtmp")
                    nc.scalar.activation(rtmp[:], den[:], AF.Ln)
                    r_bc = sm_pool.tile([128, 512], f32, tag="r_bc",
                                        name="r_bc")
                    nc.scalar.activation(r_bc[:], rtmp[:], AF.Exp,
                                         scale=-1.0)
                    nc.vector.tensor_tensor(attn[pair][:], pv[:], r_bc[:],
                                            OP.mult)
                # head-mixing projection + residual + bias
                for comp in range(2):  # 0=real, 1=imag
                    ps = pv_pool.tile([128, 512], f32,
                                      tag=("pv" if comp == 0 else "den"),
                                      name="proj")
                    nc.tensor.matmul(ps[:], lp[:, comp * 128:(comp + 1) * 128],
                                     attn[0][:], start=True, stop=False)
                    nc.tensor.matmul(ps[:],
                                     lp[:, 256 + comp * 128:384 + comp * 128],
                                     attn[1][:], start=False, stop=False)
                    nc.tensor.matmul(ps[:],
                                     pb[0:1, comp * 128:(comp + 1) * 128],
                                     ones_row[:], start=False, stop=True)
                    o_sb = out_pool.tile([128, 512], f32, tag="o_sb",
                                         name="o_sb")
                    nc.vector.tensor_tensor(o_sb[:], ps[:],
                                            resid[2 * comp + blk][:], OP.add)
                    nc.sync.dma_start(out=out_t[comp, :, bsl], in_=o_sb[:])
    split_multi_waits(nc)
    return nc


def pack_inputs(inputs):
    """Host-side exact restructuring; returns per-core input maps."""
    f = lambda k: np.asarray(inputs[k], np.float32)
    xr = f("x_real").reshape(B, C, S)
    xi = f("x_imag").reshape(B, C, S)
    Win = (f("in_w_r") + 1j * f("in_w_i")).astype(np.complex64)
    lnw = (f("ln_w_r") + 1j * f("ln_w_i")).astype(np.complex64)
    lnb = (f("ln_b_r") + 1j * f("ln_b_i")).astype(np.complex64)
    inb = (f("in_b_r") + 1j * f("in_b_i")).astype(np.complex64)
    Wp = Win * lnw[None, :]
    biasq = inb + Win @ lnb
    Wout = (f("out_w_r") + 1j * f("out_w_i")).astype(np.complex64)
    Wc = (f("conv_w_r") + 1j * f("conv_w_i")).astype(np.complex64)
    outb = (f("out_b_r") + 1j * f("out_b_i")).astype(np.complex64)
    convb = (f("conv_b_r") + 1j * f("conv_b_i")).astype(np.complex64)
    M = Wc @ Wout
    vb_ = biasq[2 * C:3 * C]
    bM = Wc @ outb + convb + M @ vb_  # v-bias folded (attn rows sum to 1)

    def pack_pair(Wsec, h0):
        W0 = Wsec[32 * h0:32 * h0 + 32]
        W1 = Wsec[32 * (h0 + 1):32 * (h0 + 1) + 32]
        RA = np.concatenate([W0.real, W0.imag, W1.real, W1.imag], 0)
        RB = np.concatenate([-W0.imag, W0.real, -W1.imag, W1.real], 0)
        return RA.T.copy(), RB.T.copy()

    qW, kW, vW = Wp[0:C], Wp[C:2 * C], Wp[2 * C:3 * C]
    qb_ = biasq[0:C]
    tiles = [pack_pair(kW, 0), pack_pair(kW, 2),
             pack_pair(qW, 0), pack_pair(qW, 2)]
    import ml_dtypes
    bf = ml_dtypes.bfloat16
    wa = np.ascontiguousarray(
        np.concatenate([t[0] for t in tiles], 1)).astype(bf)
    wb = np.ascontiguousarray(
        np.concatenate([t[1] for t in tiles], 1)).astype(bf)
    vt = [pack_pair(vW, 0), pack_pair(vW, 2)]
    wva = np.ascontiguousarray(
        np.concatenate([t[0] for t in vt], 1)).astype(bf)
    wvb = np.ascontiguousarray(
        np.concatenate([t[1] for t in vt], 1)).astype(bf)

    def pack_qbias(h0):
        b0 = qb_[32 * h0:32 * h0 + 32]
        b1 = qb_[32 * (h0 + 1):32 * (h0 + 1) + 32]
        return np.concatenate([b0.real, b0.imag, b1.real, b1.imag], 0)
    qb2 = np.ascontiguousarray(
        np.stack([pack_qbias(0), pack_qbias(2)], 1), np.float32)

    def pack_proj(h0):
        M0 = M[:, 32 * h0:32 * h0 + 32]
        M1 = M[:, 32 * (h0 + 1):32 * (h0 + 1) + 32]
        Lr = np.concatenate([M0.real.T, -M0.imag.T, M1.real.T, -M1.imag.T], 0)
        Li = np.concatenate([M0.imag.T, M0.real.T, M1.imag.T, M1.real.T], 0)
        return Lr, Li
    L01r, L01i = pack_proj(0)
    L23r, L23i = pack_proj(2)
    lp = np.ascontiguousarray(
        np.concatenate([L01r, L01i, L23r, L23i], 1)).astype(bf)
    pb = np.ascontiguousarray(
        np.concatenate([bM.real, bM.imag])[None, :]).astype(bf)

    in_maps = []
    for core in range(8):
        b, qi = core // 4, core % 4
        # permute key order: own quarter last (blocks 6-7)
        order = [q for q in range(4) if q != qi] + [qi]
        xrp = np.concatenate(
            [xr[b][:, q * SQ:(q + 1) * SQ] for q in order], 1)
        xip = np.concatenate(
            [xi[b][:, q * SQ:(q + 1) * SQ] for q in order], 1)
        in_maps.append({
            "xr": np.ascontiguousarray(xrp),
            "xi": np.ascontiguousarray(xip),
            "wa": wa, "wb": wb, "wva": wva, "wvb": wvb,
            "qb2": qb2, "lp": lp, "pb": pb,
        })
    return in_maps


_CACHED = {}


def _ensure_ntff_hook():
    """Register the axon NTFF profiling hook (absent from this image's
    antenv) so run_bass_kernel_spmd(trace=True) can capture HW timing."""
    try:
        import antenv.axon_hooks  # noqa: F401
        return
    except ImportError:
        pass
    import types

    try:
        from trn_agent_boot import trn_boot
        hook = trn_boot._ntff_profile_via_ctypes("/opt/axon/libaxon_pjrt.so")
    except Exception:
        return
    import antenv

    mod = types.ModuleType("antenv.axon_hooks")
    mod.get_axon_ntff_profile_hook = lambda: hook
    mod.set_axon_ntff_profile_hook = lambda h: None
    sys.modules["antenv.axon_hooks"] = mod
    antenv.axon_hooks = mod


def kernel(trace=False, **inputs):
    global LAST_RESULTS
    from concourse.bass_utils import run_bass_kernel_spmd

    if trace:
        _ensure_ntff_hook()

    if "nc" not in _CACHED:
        _CACHED["nc"] = build_program()
    nc = _CACHED["nc"]
    in_maps = pack_inputs(inputs)
    res = run_bass_kernel_spmd(nc, in_maps, core_ids=list(range(8)),
                               trace=trace)
    LAST_RESULTS = res
    out = np.zeros((2, B, C, S), np.float32)
    for core in range(8):
        b, qi = core // 4, core % 4
        out[:, b, :, qi * SQ:(qi + 1) * SQ] = res.results[core]["out"]
    return out.reshape(2, B, C, 64, 64)


# revision 5
# speedup vs baseline: 1.4087x; 1.2939x over previous
"""Trainium2 Bass kernel for nn_AttentionBlock (complex attention block).

Shapes: B=2, C=128, H=W=64 -> s=4096 tokens, NUM_HEADS=4, dh=32.
Sharding: 8 cores = (batch b, seq-quarter qi). Each core computes the full
attention output for 1024 query tokens of one batch element (all 4 heads),
so there are no collectives: the final channel-mixing projection is local.

Math restructuring (host-side, exact):
  - LN affine (w, b) folded into the QKV in-projection weights/bias.
  - K-bias dropped entirely (softmax is invariant to per-query shifts).
  - V-bias folded into the output-projection bias (attn rows sum to 1).
  - out-projection and 1x1 conv composed into one complex matrix
    M = Wc @ Wout; bias bM = Wc @ out_b + conv_b + M @ v_bias.
  - Key order is permuted per-core so each core's own query quarter is
    always blocks 6-7 (softmax/PV are permutation-invariant over keys),
    letting one 8-block pass produce K, V, Q and the residual.
Device pipeline per core:
  P1: stream x in 8 blocks of 512 tokens: LN stats via 1/128-ones matmul,
      bf16 elementwise chain on DVE, inv-std via exp(-0.5*ln(var+eps)),
      K projection -> ksb [feat, tok]; V projected directly token-major
      (yn_chunk^T @ Wv per 128-token chunk -> no PE transposes); on the
      last two blocks also Q projection + residual capture.
  P2: per (query-block 512, head-pair): per key chunk of 128 tokens:
      two row-tiled concurrent score matmuls (K=64 each at PE rows 0-63 /
      64-127), one exp over [128,1024] on ACT, then col-tiled PV and
      denominator matmuls accumulating into shared [128,512] PSUM banks;
      normalize with one Ln + one Exp + one multiply per pair; project
      heads with M, add residual + bias, DMA out.
"""

import os
import sys
from contextlib import ExitStack

import numpy as np

sys.path.insert(0, "/opt/trn_rl_repo")

B, C, S, SQ = 2, 128, 4096, 1024
NH, DH = 4, 32
EPS = 1e-5
SCALE = 1.0 / np.sqrt(np.float32(DH))
NB = S // 512    # 8 blocks of 512 tokens
NT = S // 128    # 32 key/value token chunks
LAST_RESULTS = None


def build_program():
    import concourse.bass as bass
    import concourse.mybir as mybir
    import concourse.tile as tile

    f32 = mybir.dt.float32
    bf16 = mybir.dt.bfloat16
    AF = mybir.ActivationFunctionType
    OP = mybir.AluOpType

    def split_multi_waits(nc):
        """walrus on this image encodes at most ONE sync wait per
        instruction; split extras into same-engine NOPs placed before."""
        def fix_block(blk):
            new_insts = []
            for inst in blk.instructions:
                try:
                    subs = inst.blocks
                except AttributeError:
                    subs = None
                if subs:
                    for sub in subs:
                        fix_block(sub)
                si = inst.sync_info
                waits = list(si.on_wait) if si is not None and si.on_wait else []
                if len(waits) > 1:
                    for j, w in enumerate(waits[:-1]):
                        nop = mybir.InstNoOp(name=f"{inst.name}-ws{j}")
                        nop.engine = inst.engine
                        nop.sync_info = mybir.SyncInfo(on_wait=[w],
                                                       on_update=[])
                        new_insts.append(nop)
                    inst.sync_info = mybir.SyncInfo(
                        on_wait=[waits[-1]], on_update=list(si.on_update))
                new_insts.append(inst)
            blk.instructions = new_insts
        for blk in nc.m.functions[0].blocks:
            fix_block(blk)

    nc = bass.Bass()

    xr_t = nc.declare_dram_parameter("xr", [C, S], f32, isOutput=False)
    xi_t = nc.declare_dram_parameter("xi", [C, S], f32, isOutput=False)
    # K/Q projection weights: [k_p0 | k_p1 | q_p0 | q_p1], 128 cols each
    wa_t = nc.declare_dram_parameter("wa", [C, 512], bf16, isOutput=False)
    wb_t = nc.declare_dram_parameter("wb", [C, 512], bf16, isOutput=False)
    # V projection (token-major output): [p0h0|p0h1|p1h0|p1h1], 64 cols each
    wva_t = nc.declare_dram_parameter("wva", [C, 256], bf16, isOutput=False)
    wvb_t = nc.declare_dram_parameter("wvb", [C, 256], bf16, isOutput=False)
    qb2_t = nc.declare_dram_parameter("qb2", [C, 2], f32, isOutput=False)
    lp_t = nc.declare_dram_parameter("lp", [C, 512], bf16, isOutput=False)
    pb_t = nc.declare_dram_parameter("pb", [1, 256], bf16, isOutput=False)
    out_t = nc.declare_dram_parameter("out", [2, C, SQ], f32, isOutput=True)

    with tile.TileContext(nc) as tc, ExitStack() as ctx:
        const_pool = ctx.enter_context(tc.tile_pool(name="const", bufs=1))
        big_pool = ctx.enter_context(tc.tile_pool(name="big", bufs=1))

        ones_bc = const_pool.tile([128, 128], bf16, tag="ones_bc", name="ones_bc")
        nc.gpsimd.memset(ones_bc[:], 1.0 / 128.0)
        ones_pv = const_pool.tile([128, 64], bf16, tag="ones_pv", name="ones_pv")
        nc.gpsimd.memset(ones_pv[:], 1.0)
        ones_row = const_pool.tile([1, 512], bf16, tag="ones_row", name="ones_row")
        nc.gpsimd.memset(ones_row[:], 1.0)
        eps_c = const_pool.tile([128, 1], f32, tag="eps_c", name="eps_c")
        nc.gpsimd.memset(eps_c[:], EPS)
        # Pre-sync ACT with gpsimd consts (and trigger the exp/ln table
        # load early) so later activations carry a single sync wait.
        act_warm = const_pool.tile([128, 1], f32, tag="act_warm",
                                   name="act_warm")
        nc.scalar.activation(act_warm[:], eps_c[:], AF.Exp)

        # First two x blocks DMA'd ahead of the weights so P1 compute can
        # start as early as possible (weights aren't needed until the
        # first K matmul, several microseconds in).
        x_pre = [const_pool.tile([128, 512], f32, tag=f"xpre{i}",
                                 name=f"xpre{i}") for i in range(4)]
        for blk in range(2):
            sl = slice(blk * 512, (blk + 1) * 512)
            nc.sync.dma_start(out=x_pre[2 * blk][:], in_=xr_t[:, sl])
            nc.sync.dma_start(out=x_pre[2 * blk + 1][:], in_=xi_t[:, sl])

        wa = const_pool.tile([C, 512], bf16, tag="wa", name="wa")
        wb = const_pool.tile([C, 512], bf16, tag="wb", name="wb")
        wva = const_pool.tile([C, 256], bf16, tag="wva", name="wva")
        wvb = const_pool.tile([C, 256], bf16, tag="wvb", name="wvb")
        qb2 = const_pool.tile([C, 2], f32, tag="qb2", name="qb2")
        lp = const_pool.tile([C, 512], bf16, tag="lp", name="lp")
        pb = const_pool.tile([1, 256], bf16, tag="pb", name="pb")
        nc.sync.dma_start(out=wa[:], in_=wa_t[:])
        nc.sync.dma_start(out=wb[:], in_=wb_t[:])
        nc.sync.dma_start(out=wva[:], in_=wva_t[:])
        nc.sync.dma_start(out=wvb[:], in_=wvb_t[:])
        nc.sync.dma_start(out=qb2[:], in_=qb2_t[:])
        nc.sync.dma_start(out=lp[:], in_=lp_t[:])
        nc.sync.dma_start(out=pb[:], in_=pb_t[:])
        # Pre-sync DVE with the first x DMA lane so the first casts carry
        # a single sync wait (walrus wait-slot limit).
        dve_warm = const_pool.tile([128, 1], f32, tag="dve_warm",
                                   name="dve_warm")
        nc.vector.tensor_copy(dve_warm[:], x_pre[0][:, 0:1])

        # persistent activation storage
        ksb = [big_pool.tile([128, S], bf16, tag=f"ksb{p}", name=f"ksb{p}")
               for p in range(2)]
        qsb = [big_pool.tile([128, SQ], bf16, tag=f"qsb{p}", name=f"qsb{p}")
               for p in range(2)]
        # vTe: token-major V, per chunk 256 cols [p0h0|p0h1|p1h0|p1h1]
        vTe = big_pool.tile([128, NT * 256], bf16, tag="vTe", name="vTe")
        resid = [big_pool.tile([128, 512], f32, tag=f"res{i}", name=f"res{i}")
                 for i in range(4)]
        # resid order: [r blk0, r blk1, i blk0, i blk1]

        # ---------------- P1: LN + QKV projection ----------------
        with ExitStack() as p1:
            xin_pool = p1.enter_context(tc.tile_pool(name="xin", bufs=6))
            tmp_pool = p1.enter_context(tc.tile_pool(name="tmp", bufs=2))
            mu_pool = p1.enter_context(
                tc.tile_pool(name="mups", bufs=1, space="PSUM"))
            var_pool = p1.enter_context(
                tc.tile_pool(name="varps", bufs=2, space="PSUM"))
            kq_pool = p1.enter_context(
                tc.tile_pool(name="kqps", bufs=2, space="PSUM"))
            vt_pool = p1.enter_context(
                tc.tile_pool(name="vtps", bufs=2, space="PSUM"))

            for blk in range(NB):
                sl = slice(blk * 512, (blk + 1) * 512)
                own = blk >= 6  # own query quarter (host permutes keys)
                qb_i = blk - 6
                if blk < 2:
                    xr_b = x_pre[2 * blk]
                    xi_b = x_pre[2 * blk + 1]
                elif own:
                    xr_b = resid[qb_i]
                    xi_b = resid[2 + qb_i]
                    nc.sync.dma_start(out=xr_b[:], in_=xr_t[:, sl])
                    nc.sync.dma_start(out=xi_b[:], in_=xi_t[:, sl])
                else:
                    xr_b = xin_pool.tile([128, 512], f32, tag="xr_b",
                                         name="xr_b")
                    xi_b = xin_pool.tile([128, 512], f32, tag="xi_b",
                                         name="xi_b")
                    nc.sync.dma_start(out=xr_b[:], in_=xr_t[:, sl])
                    nc.sync.dma_start(out=xi_b[:], in_=xi_t[:, sl])

                xb_r = tmp_pool.tile([128, 512], bf16, tag="xb_r", name="xb_r")
                xb_i = tmp_pool.tile([128, 512], bf16, tag="xb_i", name="xb_i")
                nc.vector.tensor_copy(xb_r[:], xr_b[:])
                nc.vector.tensor_copy(xb_i[:], xi_b[:])

                mu = mu_pool.tile([128, 1024], f32, tag="mu", name="mu")
                nc.tensor.matmul(mu[:, 0:512], ones_bc[:], xb_r[:],
                                 start=True, stop=True)
                nc.tensor.matmul(mu[:, 512:1024], ones_bc[:], xb_i[:],
                                 start=True, stop=True)
                mu_sb = tmp_pool.tile([128, 1024], bf16, tag="mu_sb",
                                      name="mu_sb")
                nc.scalar.activation(mu_sb[:], mu[:], AF.Copy)

                d_r = tmp_pool.tile([128, 512], bf16, tag="d_r", name="d_r")
                d_i = tmp_pool.tile([128, 512], bf16, tag="d_i", name="d_i")
                nc.vector.tensor_tensor(d_r[:], xb_r[:], mu_sb[:, 0:512],
                                        OP.subtract)
                nc.vector.tensor_tensor(d_i[:], xb_i[:], mu_sb[:, 512:1024],
                                        OP.subtract)
                sq_r = tmp_pool.tile([128, 512], bf16, tag="sq_r", name="sq_r")
                sq_i = tmp_pool.tile([128, 512], bf16, tag="sq_i", name="sq_i")
                nc.vector.tensor_tensor(sq_r[:], d_r[:], d_r[:], OP.mult)
                nc.vector.tensor_tensor(sq_i[:], d_i[:], d_i[:], OP.mult)
                var = var_pool.tile([128, 512], f32, tag="var", name="var")
                nc.tensor.matmul(var[:], ones_bc[:], sq_r[:],
                                 start=True, stop=False)
                nc.tensor.matmul(var[:], ones_bc[:], sq_i[:],
                                 start=False, stop=True)
                lntmp = tmp_pool.tile([128, 512], f32, tag="lntmp",
                                      name="lntmp")
                nc.scalar.activation(lntmp[:], var[:], AF.Ln, bias=eps_c[:])
                inv = tmp_pool.tile([128, 512], bf16, tag="inv", name="inv")
                nc.scalar.activation(inv[:], lntmp[:], AF.Exp, scale=-0.5)
                yn_r = tmp_pool.tile([128, 512], bf16, tag="yn_r", name="yn_r")
                yn_i = tmp_pool.tile([128, 512], bf16, tag="yn_i", name="yn_i")
                nc.vector.tensor_tensor(yn_r[:], d_r[:], inv[:], OP.mult)
                nc.vector.tensor_tensor(yn_i[:], d_i[:], inv[:], OP.mult)

                # K tiles (both pairs), PSUM -> ksb via ACT copy (no bias)
                for p in range(2):
                    ps = kq_pool.tile([128, 512], f32, tag="kq", name="kq")
                    nc.tensor.matmul(ps[:], wa[:, p * 128:(p + 1) * 128],
                                     yn_r[:], start=True, stop=False)
                    nc.tensor.matmul(ps[:], wb[:, p * 128:(p + 1) * 128],
                                     yn_i[:], start=False, stop=True)
                    nc.scalar.activation(ksb[p][:, sl], ps[:], AF.Copy)

                # V token-major: per 128-token chunk, yn_chunk^T @ Wv
                for cch in range(4):
                    tt = blk * 4 + cch
                    csl = slice(cch * 128, (cch + 1) * 128)
                    vt = vt_pool.tile([128, 256], f32, tag="vt", name="vt")
                    nc.tensor.matmul(vt[:], yn_r[:, csl], wva[:],
                                     start=True, stop=False)
                    nc.tensor.matmul(vt[:], yn_i[:, csl], wvb[:],
                                     start=False, stop=True)
                    nc.vector.tensor_copy(
                        vTe[:, tt * 256:(tt + 1) * 256], vt[:])

                if own:
                    qsl = slice(qb_i * 512, (qb_i + 1) * 512)
                    for p in range(2):
                        ps = kq_pool.tile([128, 512], f32, tag="kq",
                                          name="kq")
                        nc.tensor.matmul(ps[:],
                                         wa[:, 256 + p * 128:384 + p * 128],
                                         yn_r[:], start=True, stop=False)
                        nc.tensor.matmul(ps[:],
                                         wb[:, 256 + p * 128:384 + p * 128],
                                         yn_i[:], start=False, stop=True)
                        nc.vector.tensor_scalar_add(qsb[p][:, qsl], ps[:],
                                                    qb2[:, p:p + 1])

        # ---------------- P2: attention + projection ----------------
        with ExitStack() as p3:
            sc_pool = p3.enter_context(
                tc.tile_pool(name="scps", bufs=2, space="PSUM"))
            pv_pool = p3.enter_context(
                tc.tile_pool(name="pvps", bufs=2, space="PSUM"))
            exp_pool = p3.enter_context(tc.tile_pool(name="exp", bufs=3))
            sm_pool = p3.enter_context(tc.tile_pool(name="sm", bufs=2))
            out_pool = p3.enter_context(tc.tile_pool(name="outp", bufs=4))

            for blk in range(2):
                bsl = slice(blk * 512, (blk + 1) * 512)
                attn = [sm_pool.tile([128, 512], bf16, tag=f"attn{p}",
                                     name=f"attn{p}") for p in range(2)]
                for pair in range(2):
                    pv = pv_pool.tile([128, 512], f32, tag="pv", name="pv")
                    den = pv_pool.tile([128, 512], f32, tag="den", name="den")
                    exs = [None] * NT
                    # software-pipelined by one stage: scores/exp for chunk
                    # t issue before pv/den of chunk t-1, so the PE never
                    # head-of-line-blocks the next exp behind pv's wait.
                    for t in range(NT + 1):
                        if t < NT:
                            tsl = slice(t * 128, (t + 1) * 128)
                            sc = sc_pool.tile([128, 1024], f32, tag="sc",
                                              name="sc")
                            nc.tensor.matmul(sc[:, 0:512],
                                             ksb[pair][0:64, tsl],
                                             qsb[pair][0:64, bsl],
                                             start=True, stop=True)
                            nc.tensor.matmul(sc[:, 512:1024],
                                             ksb[pair][64:128, tsl],
                                             qsb[pair][64:128, bsl],
                                             start=True, stop=True,
                                             tile_position=(64, 0))
                            ex = exp_pool.tile([128, 1024], bf16, tag="ex",
                                               name="ex")
                            nc.scalar.activation(ex[:], sc[:], AF.Exp,
                                                 scale=float(SCALE))
                            exs[t] = ex
                        if t >= 1:
                            tt = t - 1
                            ex = exs[tt]
                            exs[tt] = None
                            st, sp = tt == 0, tt == NT - 1
                            vb = tt * 256 + pair * 128
                            # col-tiled cross pairs: (pv_h0 | den_h1) and
                            # (pv_h1 | den_h0) run concurrently on the PE
                            nc.tensor.matmul(pv[0:64, :],
                                             vTe[:, vb:vb + 64],
                                             ex[:, 0:512],
                                             start=st, stop=sp,
                                             skip_group_check=True)
                            nc.tensor.matmul(den[64:128, :], ones_pv[:],
                                             ex[:, 512:1024],
                                             start=st, stop=sp,
                                             tile_position=(0, 64),
                                             skip_group_check=True)
                            nc.tensor.matmul(pv[64:128, :],
                                             vTe[:, vb + 64:vb + 128],
                                             ex[:, 512:1024],
                                             start=False, stop=sp,
                                             tile_position=(0, 64),
                                             skip_group_check=True)
                            nc.tensor.matmul(den[0:64, :], ones_pv[:],
                                             ex[:, 0:512],
                                             start=False, stop=sp,
                                             skip_group_check=True)
                    rtmp = sm_pool.tile([128, 512], f32, tag="rtmp",
                                        name="rtmp")
                    nc.scalar.activation(rtmp[:], den[:], AF.Ln)
                    r_bc = sm_pool.tile([128, 512], f32, tag="r_bc",
                                        name="r_bc")
                    nc.scalar.activation(r_bc[:], rtmp[:], AF.Exp,
                                         scale=-1.0)
                    nc.vector.tensor_tensor(attn[pair][:], pv[:], r_bc[:],
                                            OP.mult)
                # head-mixing projection + residual + bias
                for comp in range(2):  # 0=real, 1=imag
                    ps = pv_pool.tile([128, 512], f32,
                                      tag=("pv" if comp == 0 else "den"),
                                      name="proj")
                    nc.tensor.matmul(ps[:], lp[:, comp * 128:(comp + 1) * 128],
                                     attn[0][:], start=True, stop=False)
                    nc.tensor.matmul(ps[:],
                                     lp[:, 256 + comp * 128:384 + comp * 128],
                                     attn[1][:], start=False, stop=False)
                    nc.tensor.matmul(ps[:],
                                     pb[0:1, comp * 128:(comp + 1) * 128],
                                     ones_row[:], start=False, stop=True)
                    o_sb = out_pool.tile([128, 512], f32, tag="o_sb",
                                         name="o_sb")
                    nc.vector.tensor_tensor(o_sb[:], ps[:],
                                            resid[2 * comp + blk][:], OP.add)
                    nc.sync.dma_start(out=out_t[comp, :, bsl], in_=o_sb[:])
    split_multi_waits(nc)
    return nc


def pack_inputs(inputs):
    """Host-side exact restructuring; returns per-core input maps."""
    f = lambda k: np.asarray(inputs[k], np.float32)
    xr = f("x_real").reshape(B, C, S)
    xi = f("x_imag").reshape(B, C, S)
    Win = (f("in_w_r") + 1j * f("in_w_i")).astype(np.complex64)
    lnw = (f("ln_w_r") + 1j * f("ln_w_i")).astype(np.complex64)
    lnb = (f("ln_b_r") + 1j * f("ln_b_i")).astype(np.complex64)
    inb = (f("in_b_r") + 1j * f("in_b_i")).astype(np.complex64)
    Wp = Win * lnw[None, :]
    biasq = inb + Win @ lnb
    Wout = (f("out_w_r") + 1j * f("out_w_i")).astype(np.complex64)
    Wc = (f("conv_w_r") + 1j * f("conv_w_i")).astype(np.complex64)
    outb = (f("out_b_r") + 1j * f("out_b_i")).astype(np.complex64)
    convb = (f("conv_b_r") + 1j * f("conv_b_i")).astype(np.complex64)
    M = Wc @ Wout
    vb_ = biasq[2 * C:3 * C]
    bM = Wc @ outb + convb + M @ vb_  # v-bias folded (attn rows sum to 1)

    def pack_pair(Wsec, h0):
        W0 = Wsec[32 * h0:32 * h0 + 32]
        W1 = Wsec[32 * (h0 + 1):32 * (h0 + 1) + 32]
        RA = np.concatenate([W0.real, W0.imag, W1.real, W1.imag], 0)
        RB = np.concatenate([-W0.imag, W0.real, -W1.imag, W1.real], 0)
        return RA.T.copy(), RB.T.copy()

    qW, kW, vW = Wp[0:C], Wp[C:2 * C], Wp[2 * C:3 * C]
    qb_ = biasq[0:C]
    tiles = [pack_pair(kW, 0), pack_pair(kW, 2),
             pack_pair(qW, 0), pack_pair(qW, 2)]
    import ml_dtypes
    bf = ml_dtypes.bfloat16
    wa = np.ascontiguousarray(
        np.concatenate([t[0] for t in tiles], 1)).astype(bf)
    wb = np.ascontiguousarray(
        np.concatenate([t[1] for t in tiles], 1)).astype(bf)
    vt = [pack_pair(vW, 0), pack_pair(vW, 2)]
    wva = np.ascontiguousarray(
        np.concatenate([t[0] for t in vt], 1)).astype(bf)
    wvb = np.ascontiguousarray(
        np.concatenate([t[1] for t in vt], 1)).astype(bf)

    def pack_qbias(h0):
        b0 = qb_[32 * h0:32 * h0 + 32]
        b1 = qb_[32 * (h0 + 1):32 * (h0 + 1) + 32]
        return np.concatenate([b0.real, b0.imag, b1.real, b1.imag], 0)
    qb2 = np.ascontiguousarray(
        np.stack([pack_qbias(0), pack_qbias(2)], 1), np.float32)

    def pack_proj(h0):
        M0 = M[:, 32 * h0:32 * h0 + 32]
        M1 = M[:, 32 * (h0 + 1):32 * (h0 + 1) + 32]
        Lr = np.concatenate([M0.real.T, -M0.imag.T, M1.real.T, -M1.imag.T], 0)
        Li = np.concatenate([M0.imag.T, M0.real.T, M1.imag.T, M1.real.T], 0)
        return Lr, Li
    L01r, L01i = pack_proj(0)
    L23r, L23i = pack_proj(2)
    lp = np.ascontiguousarray(
        np.concatenate([L01r, L01i, L23r, L23i], 1)).astype(bf)
    pb = np.ascontiguousarray(
        np.concatenate([bM.real, bM.imag])[None, :]).astype(bf)

    in_maps = []
    for core in range(8):
        b, qi = core // 4, core % 4
        # permute key order: own quarter last (blocks 6-7)
        order = [q for q in range(4) if q != qi] + [qi]
        xrp = np.concatenate(
            [xr[b][:, q * SQ:(q + 1) * SQ] for q in order], 1)
        xip = np.concatenate(
            [xi[b][:, q * SQ:(q + 1) * SQ] for q in order], 1)
        in_maps.append({
            "xr": np.ascontiguousarray(xrp),
            "xi": np.ascontiguousarray(xip),
            "wa": wa, "wb": wb, "wva": wva, "wvb": wvb,
            "qb2": qb2, "lp": lp, "pb": pb,
        })
    return in_maps


_CACHED = {}


def _ensure_ntff_hook():
    """Register the axon NTFF profiling hook (absent from this image's
    antenv) so run_bass_kernel_spmd(trace=True) can capture HW timing."""
    try:
        import antenv.axon_hooks  # noqa: F401
        return
    except ImportError:
        pass
    import types

    try:
        from trn_agent_boot import trn_boot
        hook = trn_boot._ntff_profile_via_ctypes("/opt/axon/libaxon_pjrt.so")
    except Exception:
        return
    import antenv

    mod = types.ModuleType("antenv.axon_hooks")
    mod.get_axon_ntff_profile_hook = lambda: hook
    mod.set_axon_ntff_profile_hook = lambda h: None
    sys.modules["antenv.axon_hooks"] = mod
    antenv.axon_hooks = mod


def kernel(trace=False, **inputs):
    global LAST_RESULTS
    from concourse.bass_utils import run_bass_kernel_spmd

    if trace:
        _ensure_ntff_hook()

    if "nc" not in _CACHED:
        _CACHED["nc"] = build_program()
    nc = _CACHED["nc"]
    in_maps = pack_inputs(inputs)
    res = run_bass_kernel_spmd(nc, in_maps, core_ids=list(range(8)),
                               trace=trace)
    LAST_RESULTS = res
    out = np.zeros((2, B, C, S), np.float32)
    for core in range(8):
        b, qi = core // 4, core % 4
        out[:, b, :, qi * SQ:(qi + 1) * SQ] = res.results[core]["out"]
    return out.reshape(2, B, C, 64, 64)


# revision 6
# speedup vs baseline: 1.4420x; 1.0236x over previous
"""Trainium2 Bass kernel for nn_AttentionBlock (complex attention block).

Shapes: B=2, C=128, H=W=64 -> s=4096 tokens, NUM_HEADS=4, dh=32.
Sharding: 8 cores = (batch b, seq-quarter qi). Each core computes the full
attention output for 1024 query tokens of one batch element (all 4 heads),
so there are no collectives: the final channel-mixing projection is local.

Math restructuring (host-side, exact):
  - LN affine (w, b) folded into the QKV in-projection weights/bias.
  - K-bias dropped entirely (softmax is invariant to per-query shifts).
  - V-bias folded into the output-projection bias (attn rows sum to 1).
  - Projection bias folded into the residual tiles (per-channel column).
  - x pre-cast to bf16 host-side (residual kept f32 via a small side DMA).
  - Key order is permuted per-core so each core's own query quarter is
    always blocks 6-7 (softmax/PV are permutation-invariant over keys),
    letting one 8-block pass produce K, V, Q and the residual.
Device pipeline per core:
  P1 (software-pipelined stages A/B/C across 512-token blocks):
      A: DMA x block, LN mean via 1/128-ones matmul, mean PSUM->SBUF (ACT)
      B: center + square (DVE bf16), var matmul, inv-std via exp(-0.5*ln)
      C: yn = d*inv; K projection -> ksb [feat, tok] (ACT copy-out);
         V projected directly token-major (yn_chunk^T @ Wv, no transposes);
         blocks 6-7 also produce Q (+bias) and the residual.
  P2: per (query-block 512, head-pair): scores for 64 (chunk, head) units
      in granules of 3 -> one exp[128,1536] ACT call per granule; two
      row-tiled concurrent score matmuls per chunk (K=64 at PE rows 0-63 /
      64-127); col-tiled cross-paired PV/denominator matmuls accumulate
      into shared [128,512] PSUM banks one granule behind the exp stream;
      normalize with one Ln + one Exp + one multiply per pair; projections
      deferred to overlap the next pair's score stream.
"""

import os
import sys
from contextlib import ExitStack

import numpy as np

sys.path.insert(0, "/opt/trn_rl_repo")

B, C, S, SQ = 2, 128, 4096, 1024
NH, DH = 4, 32
EPS = 1e-5
SCALE = 1.0 / np.sqrt(np.float32(DH))
NB = S // 512    # 8 blocks of 512 tokens
NT = S // 128    # 32 key/value token chunks
LAST_RESULTS = None


def build_program():
    import concourse.bass as bass
    import concourse.mybir as mybir
    import concourse.tile as tile

    f32 = mybir.dt.float32
    bf16 = mybir.dt.bfloat16
    AF = mybir.ActivationFunctionType
    OP = mybir.AluOpType

    def split_multi_waits(nc):
        """walrus on this image encodes at most ONE sync wait per
        instruction; split extras into same-engine NOPs placed before."""
        def fix_block(blk):
            new_insts = []
            for inst in blk.instructions:
                try:
                    subs = inst.blocks
                except AttributeError:
                    subs = None
                if subs:
                    for sub in subs:
                        fix_block(sub)
                si = inst.sync_info
                waits = list(si.on_wait) if si is not None and si.on_wait else []
                if len(waits) > 1:
                    for j, w in enumerate(waits[:-1]):
                        nop = mybir.InstNoOp(name=f"{inst.name}-ws{j}")
                        nop.engine = inst.engine
                        nop.sync_info = mybir.SyncInfo(on_wait=[w],
                                                       on_update=[])
                        new_insts.append(nop)
                    inst.sync_info = mybir.SyncInfo(
                        on_wait=[waits[-1]], on_update=list(si.on_update))
                new_insts.append(inst)
            blk.instructions = new_insts
        for blk in nc.m.functions[0].blocks:
            fix_block(blk)

    nc = bass.Bass()

    xr_t = nc.declare_dram_parameter("xr", [C, S], bf16, isOutput=False)
    xi_t = nc.declare_dram_parameter("xi", [C, S], bf16, isOutput=False)
    # f32 residual source: own quarter only
    rqr_t = nc.declare_dram_parameter("rqr", [C, SQ], f32, isOutput=False)
    rqi_t = nc.declare_dram_parameter("rqi", [C, SQ], f32, isOutput=False)
    # K/Q projection weights: [k_p0 | k_p1 | q_p0 | q_p1], 128 cols each
    wa_t = nc.declare_dram_parameter("wa", [C, 512], bf16, isOutput=False)
    wb_t = nc.declare_dram_parameter("wb", [C, 512], bf16, isOutput=False)
    # V projection (token-major output): [p0h0|p0h1|p1h0|p1h1], 64 cols each
    wva_t = nc.declare_dram_parameter("wva", [C, 256], bf16, isOutput=False)
    wvb_t = nc.declare_dram_parameter("wvb", [C, 256], bf16, isOutput=False)
    qb2_t = nc.declare_dram_parameter("qb2", [C, 2], f32, isOutput=False)
    lp_t = nc.declare_dram_parameter("lp", [C, 512], bf16, isOutput=False)
    pbf_t = nc.declare_dram_parameter("pbf", [C, 2], f32, isOutput=False)
    out_t = nc.declare_dram_parameter("out", [2, C, SQ], f32, isOutput=True)

    with tile.TileContext(nc) as tc, ExitStack() as ctx:
        const_pool = ctx.enter_context(tc.tile_pool(name="const", bufs=1))
        big_pool = ctx.enter_context(tc.tile_pool(name="big", bufs=1))

        # First two x blocks DMA'd ahead of the weights so P1 compute can
        # start as early as possible.
        x_pre = [const_pool.tile([128, 512], bf16, tag=f"xpre{i}",
                                 name=f"xpre{i}") for i in range(4)]
        for blk in range(2):
            sl = slice(blk * 512, (blk + 1) * 512)
            nc.sync.dma_start(out=x_pre[2 * blk][:], in_=xr_t[:, sl])
            nc.sync.dma_start(out=x_pre[2 * blk + 1][:], in_=xi_t[:, sl])

        ones_bc = const_pool.tile([128, 128], bf16, tag="ones_bc", name="ones_bc")
        nc.gpsimd.memset(ones_bc[:], 1.0 / 128.0)
        ones_pv = const_pool.tile([128, 64], bf16, tag="ones_pv", name="ones_pv")
        nc.gpsimd.memset(ones_pv[:], 1.0)
        eps_c = const_pool.tile([128, 1], f32, tag="eps_c", name="eps_c")
        nc.gpsimd.memset(eps_c[:], EPS)
        # Pre-sync ACT with gpsimd consts (and trigger the exp/ln table
        # load early) so later activations carry a single sync wait.
        act_warm = const_pool.tile([128, 1], f32, tag="act_warm",
                                   name="act_warm")
        nc.scalar.activation(act_warm[:], eps_c[:], AF.Exp)

        wa = const_pool.tile([C, 512], bf16, tag="wa", name="wa")
        wb = const_pool.tile([C, 512], bf16, tag="wb", name="wb")
        wva = const_pool.tile([C, 256], bf16, tag="wva", name="wva")
        wvb = const_pool.tile([C, 256], bf16, tag="wvb", name="wvb")
        qb2 = const_pool.tile([C, 2], f32, tag="qb2", name="qb2")
        lp = const_pool.tile([C, 512], bf16, tag="lp", name="lp")
        pbf = const_pool.tile([C, 2], f32, tag="pbf", name="pbf")
        nc.sync.dma_start(out=wa[:], in_=wa_t[:])
        nc.sync.dma_start(out=wb[:], in_=wb_t[:])
        nc.sync.dma_start(out=wva[:], in_=wva_t[:])
        nc.sync.dma_start(out=wvb[:], in_=wvb_t[:])
        nc.sync.dma_start(out=qb2[:], in_=qb2_t[:])
        nc.sync.dma_start(out=lp[:], in_=lp_t[:])
        nc.sync.dma_start(out=pbf[:], in_=pbf_t[:])
        # Pre-sync DVE with the first x DMA lane so the first DVE ops carry
        # a single sync wait (walrus wait-slot limit).
        dve_warm = const_pool.tile([128, 1], bf16, tag="dve_warm",
                                   name="dve_warm")
        nc.vector.tensor_copy(dve_warm[:], x_pre[0][:, 0:1])

        # persistent activation storage
        ksb = [big_pool.tile([128, S], bf16, tag=f"ksb{p}", name=f"ksb{p}")
               for p in range(2)]
        qsb = [big_pool.tile([128, SQ], bf16, tag=f"qsb{p}", name=f"qsb{p}")
               for p in range(2)]
        # vTe: token-major V, per chunk 256 cols [p0h0|p0h1|p1h0|p1h1]
        vTe = big_pool.tile([128, NT * 256], bf16, tag="vTe", name="vTe")
        resid = [big_pool.tile([128, 512], f32, tag=f"res{i}", name=f"res{i}")
                 for i in range(4)]
        # resid order: [r blk0, r blk1, i blk0, i blk1]

        # ---------------- P1: LN + QKV projection ----------------
        with ExitStack() as p1:
            xin_pool = p1.enter_context(tc.tile_pool(name="xin", bufs=6))
            tmp_pool = p1.enter_context(tc.tile_pool(name="tmp", bufs=2))
            mu_pool = p1.enter_context(
                tc.tile_pool(name="mups", bufs=1, space="PSUM"))
            var_pool = p1.enter_context(
                tc.tile_pool(name="varps", bufs=2, space="PSUM"))
            kq_pool = p1.enter_context(
                tc.tile_pool(name="kqps", bufs=2, space="PSUM"))
            vt_pool = p1.enter_context(
                tc.tile_pool(name="vtps", bufs=2, space="PSUM"))

            st_ = [dict() for _ in range(NB)]

            def stage_a(b):
                s = st_[b]
                sl = slice(b * 512, (b + 1) * 512)
                own = b >= 6
                qb_i = b - 6
                if b < 2:
                    xb_r, xb_i = x_pre[2 * b], x_pre[2 * b + 1]
                else:
                    xb_r = xin_pool.tile([128, 512], bf16, tag="xb_r",
                                         name="xb_r")
                    xb_i = xin_pool.tile([128, 512], bf16, tag="xb_i",
                                         name="xb_i")
                    nc.sync.dma_start(out=xb_r[:], in_=xr_t[:, sl])
                    nc.sync.dma_start(out=xb_i[:], in_=xi_t[:, sl])
                if own:
                    qsl = slice(qb_i * 512, (qb_i + 1) * 512)
                    rr, ri = resid[qb_i], resid[2 + qb_i]
                    nc.sync.dma_start(out=rr[:], in_=rqr_t[:, qsl])
                    nc.sync.dma_start(out=ri[:], in_=rqi_t[:, qsl])
                    # fold projection bias into the residual (per channel)
                    nc.vector.tensor_scalar_add(rr[:], rr[:], pbf[:, 0:1])
                    nc.vector.tensor_scalar_add(ri[:], ri[:], pbf[:, 1:2])
                mu = mu_pool.tile([128, 1024], f32, tag="mu", name="mu")
                nc.tensor.matmul(mu[:, 0:512], ones_bc[:], xb_r[:],
                                 start=True, stop=True)
                nc.tensor.matmul(mu[:, 512:1024], ones_bc[:], xb_i[:],
                                 start=True, stop=True)
                mu_sb = tmp_pool.tile([128, 1024], bf16, tag="mu_sb",
                                      name="mu_sb")
                nc.scalar.activation(mu_sb[:], mu[:], AF.Copy)
                s["xb"] = (xb_r, xb_i)
                s["mu_sb"] = mu_sb

            def stage_b(b):
                s = st_[b]
                xb_r, xb_i = s["xb"]
                mu_sb = s["mu_sb"]
                d_r = tmp_pool.tile([128, 512], bf16, tag="d_r", name="d_r")
                d_i = tmp_pool.tile([128, 512], bf16, tag="d_i", name="d_i")
                nc.vector.tensor_tensor(d_r[:], xb_r[:], mu_sb[:, 0:512],
                                        OP.subtract)
                nc.vector.tensor_tensor(d_i[:], xb_i[:], mu_sb[:, 512:1024],
                                        OP.subtract)
                sq_r = tmp_pool.tile([128, 512], bf16, tag="sq_r", name="sq_r")
                sq_i = tmp_pool.tile([128, 512], bf16, tag="sq_i", name="sq_i")
                nc.vector.tensor_tensor(sq_r[:], d_r[:], d_r[:], OP.mult)
                nc.vector.tensor_tensor(sq_i[:], d_i[:], d_i[:], OP.mult)
                var = var_pool.tile([128, 512], f32, tag="var", name="var")
                nc.tensor.matmul(var[:], ones_bc[:], sq_r[:],
                                 start=True, stop=False)
                nc.tensor.matmul(var[:], ones_bc[:], sq_i[:],
                                 start=False, stop=True)
                lntmp = tmp_pool.tile([128, 512], f32, tag="lntmp",
                                      name="lntmp")
                nc.scalar.activation(lntmp[:], var[:], AF.Ln, bias=eps_c[:])
                inv = tmp_pool.tile([128, 512], bf16, tag="inv", name="inv")
                nc.scalar.activation(inv[:], lntmp[:], AF.Exp, scale=-0.5)
                s["d"] = (d_r, d_i)
                s["inv"] = inv

            def stage_c(b):
                s = st_[b]
                sl = slice(b * 512, (b + 1) * 512)
                own = b >= 6
                qb_i = b - 6
                d_r, d_i = s["d"]
                inv = s["inv"]
                yn_r = tmp_pool.tile([128, 512], bf16, tag="yn_r", name="yn_r")
                yn_i = tmp_pool.tile([128, 512], bf16, tag="yn_i", name="yn_i")
                nc.vector.tensor_tensor(yn_r[:], d_r[:], inv[:], OP.mult)
                nc.vector.tensor_tensor(yn_i[:], d_i[:], inv[:], OP.mult)

                # K tiles (both pairs), PSUM -> ksb via ACT copy (no bias)
                for p in range(2):
                    ps = kq_pool.tile([128, 512], f32, tag="kq", name="kq")
                    nc.tensor.matmul(ps[:], wa[:, p * 128:(p + 1) * 128],
                                     yn_r[:], start=True, stop=False)
                    nc.tensor.matmul(ps[:], wb[:, p * 128:(p + 1) * 128],
                                     yn_i[:], start=False, stop=True)
                    nc.scalar.activation(ksb[p][:, sl], ps[:], AF.Copy)

                # V token-major: per 128-token chunk, yn_chunk^T @ Wv
                for cch in range(4):
                    tt = b * 4 + cch
                    csl = slice(cch * 128, (cch + 1) * 128)
                    vt = vt_pool.tile([128, 256], f32, tag="vt", name="vt")
                    nc.tensor.matmul(vt[:], yn_r[:, csl], wva[:],
                                     start=True, stop=False)
                    nc.tensor.matmul(vt[:], yn_i[:, csl], wvb[:],
                                     start=False, stop=True)
                    nc.vector.tensor_copy(
                        vTe[:, tt * 256:(tt + 1) * 256], vt[:])

                if own:
                    qsl = slice(qb_i * 512, (qb_i + 1) * 512)
                    for p in range(2):
                        ps = kq_pool.tile([128, 512], f32, tag="kq",
                                          name="kq")
                        nc.tensor.matmul(ps[:],
                                         wa[:, 256 + p * 128:384 + p * 128],
                                         yn_r[:], start=True, stop=False)
                        nc.tensor.matmul(ps[:],
                                         wb[:, 256 + p * 128:384 + p * 128],
                                         yn_i[:], start=False, stop=True)
                        nc.vector.tensor_scalar_add(qsb[p][:, qsl], ps[:],
                                                    qb2[:, p:p + 1])

            for t in range(NB + 2):
                if t < NB:
                    stage_a(t)
                if 1 <= t <= NB:
                    stage_b(t - 1)
                if t >= 2:
                    stage_c(t - 2)

        # ---------------- P2: attention + projection ----------------
        with ExitStack() as p3:
            sc_pool = p3.enter_context(
                tc.tile_pool(name="scps", bufs=2, space="PSUM"))
            pv_pool = p3.enter_context(
                tc.tile_pool(name="pvps", bufs=1, space="PSUM"))
            exp_pool = p3.enter_context(tc.tile_pool(name="exp", bufs=3))
            sm_pool = p3.enter_context(tc.tile_pool(name="sm", bufs=2))
            out_pool = p3.enter_context(tc.tile_pool(name="outp", bufs=4))

            NU = 2 * NT  # (chunk, head) score units per (qblock, pair)
            grans = [list(range(g * 3, min(g * 3 + 3, NU)))
                     for g in range((NU + 2) // 3)]
            attn_t = {}

            def attn_pair(blk, pair):
                bsl = slice(blk * 512, (blk + 1) * 512)
                pv = pv_pool.tile([128, 512], f32, tag="pv", name="pv")
                den = pv_pool.tile([128, 512], f32, tag="den", name="den")
                ex_of = {}
                next_chunk = 0
                for gi in range(len(grans) + 1):
                    if gi < len(grans):
                        units = grans[gi]
                        w = 512 * len(units)
                        sc = sc_pool.tile([128, 1536], f32, tag="sc",
                                          name="sc")
                        for j, u in enumerate(units):
                            tt, h = u // 2, u % 2
                            tsl = slice(tt * 128, (tt + 1) * 128)
                            co = j * 512
                            if h == 0:
                                nc.tensor.matmul(sc[:, co:co + 512],
                                                 ksb[pair][0:64, tsl],
                                                 qsb[pair][0:64, bsl],
                                                 start=True, stop=True)
                            else:
                                nc.tensor.matmul(sc[:, co:co + 512],
                                                 ksb[pair][64:128, tsl],
                                                 qsb[pair][64:128, bsl],
                                                 start=True, stop=True,
                                                 tile_position=(64, 0))
                        ex = exp_pool.tile([128, 1536], bf16, tag="ex",
                                           name="ex")
                        nc.scalar.activation(ex[:, 0:w], sc[:, 0:w], AF.Exp,
                                             scale=float(SCALE))
                        for j, u in enumerate(units):
                            ex_of[u] = (ex, j * 512)
                    ready = grans[gi - 1][-1] if gi >= 1 else -1
                    while (next_chunk < NT
                           and 2 * next_chunk + 1 <= ready):
                        tt = next_chunk
                        ex0, c0 = ex_of.pop(2 * tt)
                        ex1, c1 = ex_of.pop(2 * tt + 1)
                        st, sp = tt == 0, tt == NT - 1
                        vb = tt * 256 + pair * 128
                        # col-tiled cross pairs: (pv_h0 | den_h1) and
                        # (pv_h1 | den_h0) run concurrently on the PE
                        nc.tensor.matmul(pv[0:64, :],
                                         vTe[:, vb:vb + 64],
                                         ex0[:, c0:c0 + 512],
                                         start=st, stop=sp,
                                         skip_group_check=True)
                        nc.tensor.matmul(den[64:128, :], ones_pv[:],
                                         ex1[:, c1:c1 + 512],
                                         start=st, stop=sp,
                                         tile_position=(0, 64),
                                         skip_group_check=True)
                        nc.tensor.matmul(pv[64:128, :],
                                         vTe[:, vb + 64:vb + 128],
                                         ex1[:, c1:c1 + 512],
                                         start=False, stop=sp,
                                         tile_position=(0, 64),
                                         skip_group_check=True)
                        nc.tensor.matmul(den[0:64, :], ones_pv[:],
                                         ex0[:, c0:c0 + 512],
                                         start=False, stop=sp,
                                         skip_group_check=True)
                        next_chunk += 1
                rtmp = sm_pool.tile([128, 512], f32, tag="rtmp", name="rtmp")
                nc.scalar.activation(rtmp[:], den[:], AF.Ln)
                r_bc = sm_pool.tile([128, 512], f32, tag="r_bc", name="r_bc")
                nc.scalar.activation(r_bc[:], rtmp[:], AF.Exp, scale=-1.0)
                at = sm_pool.tile([128, 512], bf16, tag=f"attn{pair}",
                                  name=f"attn{pair}")
                nc.vector.tensor_tensor(at[:], pv[:], r_bc[:], OP.mult)
                attn_t[(blk, pair)] = at

            def proj(blk):
                bsl = slice(blk * 512, (blk + 1) * 512)
                for comp in range(2):  # 0=real, 1=imag
                    ps = pv_pool.tile([128, 512], f32,
                                      tag=("pv" if comp == 0 else "den"),
                                      name="proj")
                    nc.tensor.matmul(ps[:], lp[:, comp * 128:(comp + 1) * 128],
                                     attn_t[(blk, 0)][:],
                                     start=True, stop=False)
                    nc.tensor.matmul(ps[:],
                                     lp[:, 256 + comp * 128:384 + comp * 128],
                                     attn_t[(blk, 1)][:],
                                     start=False, stop=True)
                    o_sb = out_pool.tile([128, 512], f32, tag="o_sb",
                                         name="o_sb")
                    nc.vector.tensor_tensor(o_sb[:], ps[:],
                                            resid[2 * comp + blk][:], OP.add)
                    nc.sync.dma_start(out=out_t[comp, :, bsl], in_=o_sb[:])

            attn_pair(0, 0)
            attn_pair(0, 1)
            attn_pair(1, 0)
            proj(0)
            attn_pair(1, 1)
            proj(1)
    split_multi_waits(nc)
    return nc


def pack_inputs(inputs):
    """Host-side exact restructuring; returns per-core input maps."""
    f = lambda k: np.asarray(inputs[k], np.float32)
    xr = f("x_real").reshape(B, C, S)
    xi = f("x_imag").reshape(B, C, S)
    Win = (f("in_w_r") + 1j * f("in_w_i")).astype(np.complex64)
    lnw = (f("ln_w_r") + 1j * f("ln_w_i")).astype(np.complex64)
    lnb = (f("ln_b_r") + 1j * f("ln_b_i")).astype(np.complex64)
    inb = (f("in_b_r") + 1j * f("in_b_i")).astype(np.complex64)
    Wp = Win * lnw[None, :]
    biasq = inb + Win @ lnb
    Wout = (f("out_w_r") + 1j * f("out_w_i")).astype(np.complex64)
    Wc = (f("conv_w_r") + 1j * f("conv_w_i")).astype(np.complex64)
    outb = (f("out_b_r") + 1j * f("out_b_i")).astype(np.complex64)
    convb = (f("conv_b_r") + 1j * f("conv_b_i")).astype(np.complex64)
    M = Wc @ Wout
    vb_ = biasq[2 * C:3 * C]
    bM = Wc @ outb + convb + M @ vb_  # v-bias folded (attn rows sum to 1)

    def pack_pair(Wsec, h0):
        W0 = Wsec[32 * h0:32 * h0 + 32]
        W1 = Wsec[32 * (h0 + 1):32 * (h0 + 1) + 32]
        RA = np.concatenate([W0.real, W0.imag, W1.real, W1.imag], 0)
        RB = np.concatenate([-W0.imag, W0.real, -W1.imag, W1.real], 0)
        return RA.T.copy(), RB.T.copy()

    qW, kW, vW = Wp[0:C], Wp[C:2 * C], Wp[2 * C:3 * C]
    qb_ = biasq[0:C]
    tiles = [pack_pair(kW, 0), pack_pair(kW, 2),
             pack_pair(qW, 0), pack_pair(qW, 2)]
    import ml_dtypes
    bf = ml_dtypes.bfloat16
    wa = np.ascontiguousarray(
        np.concatenate([t[0] for t in tiles], 1)).astype(bf)
    wb = np.ascontiguousarray(
        np.concatenate([t[1] for t in tiles], 1)).astype(bf)
    vt = [pack_pair(vW, 0), pack_pair(vW, 2)]
    wva = np.ascontiguousarray(
        np.concatenate([t[0] for t in vt], 1)).astype(bf)
    wvb = np.ascontiguousarray(
        np.concatenate([t[1] for t in vt], 1)).astype(bf)

    def pack_qbias(h0):
        b0 = qb_[32 * h0:32 * h0 + 32]
        b1 = qb_[32 * (h0 + 1):32 * (h0 + 1) + 32]
        return np.concatenate([b0.real, b0.imag, b1.real, b1.imag], 0)
    qb2 = np.ascontiguousarray(
        np.stack([pack_qbias(0), pack_qbias(2)], 1), np.float32)

    def pack_proj(h0):
        M0 = M[:, 32 * h0:32 * h0 + 32]
        M1 = M[:, 32 * (h0 + 1):32 * (h0 + 1) + 32]
        Lr = np.concatenate([M0.real.T, -M0.imag.T, M1.real.T, -M1.imag.T], 0)
        Li = np.concatenate([M0.imag.T, M0.real.T, M1.imag.T, M1.real.T], 0)
        return Lr, Li
    L01r, L01i = pack_proj(0)
    L23r, L23i = pack_proj(2)
    lp = np.ascontiguousarray(
        np.concatenate([L01r, L01i, L23r, L23i], 1)).astype(bf)
    pbf = np.ascontiguousarray(
        np.stack([bM.real, bM.imag], 1), np.float32)

    xr16 = xr.astype(bf)
    xi16 = xi.astype(bf)
    in_maps = []
    for core in range(8):
        b, qi = core // 4, core % 4
        # permute key order: own quarter last (blocks 6-7)
        order = [q for q in range(4) if q != qi] + [qi]
        xrp = np.concatenate(
            [xr16[b][:, q * SQ:(q + 1) * SQ] for q in order], 1)
        xip = np.concatenate(
            [xi16[b][:, q * SQ:(q + 1) * SQ] for q in order], 1)
        qsl = slice(qi * SQ, (qi + 1) * SQ)
        in_maps.append({
            "xr": np.ascontiguousarray(xrp),
            "xi": np.ascontiguousarray(xip),
            "rqr": np.ascontiguousarray(xr[b][:, qsl]),
            "rqi": np.ascontiguousarray(xi[b][:, qsl]),
            "wa": wa, "wb": wb, "wva": wva, "wvb": wvb,
            "qb2": qb2, "lp": lp, "pbf": pbf,
        })
    return in_maps


_CACHED = {}


def _ensure_ntff_hook():
    """Register the axon NTFF profiling hook (absent from this image's
    antenv) so run_bass_kernel_spmd(trace=True) can capture HW timing."""
    try:
        import antenv.axon_hooks  # noqa: F401
        return
    except ImportError:
        pass
    import types

    try:
        from trn_agent_boot import trn_boot
        hook = trn_boot._ntff_profile_via_ctypes("/opt/axon/libaxon_pjrt.so")
    except Exception:
        return
    import antenv

    mod = types.ModuleType("antenv.axon_hooks")
    mod.get_axon_ntff_profile_hook = lambda: hook
    mod.set_axon_ntff_profile_hook = lambda h: None
    sys.modules["antenv.axon_hooks"] = mod
    antenv.axon_hooks = mod


def kernel(trace=False, **inputs):
    global LAST_RESULTS
    from concourse.bass_utils import run_bass_kernel_spmd

    if trace:
        _ensure_ntff_hook()

    if "nc" not in _CACHED:
        _CACHED["nc"] = build_program()
    nc = _CACHED["nc"]
    in_maps = pack_inputs(inputs)
    res = run_bass_kernel_spmd(nc, in_maps, core_ids=list(range(8)),
                               trace=trace)
    LAST_RESULTS = res
    out = np.zeros((2, B, C, S), np.float32)
    for core in range(8):
        b, qi = core // 4, core % 4
        out[:, b, :, qi * SQ:(qi + 1) * SQ] = res.results[core]["out"]
    return out.reshape(2, B, C, 64, 64)


# revision 9
# speedup vs baseline: 1.4438x; 1.0013x over previous
"""Trainium2 Bass kernel for nn_AttentionBlock (complex attention block).

Shapes: B=2, C=128, H=W=64 -> s=4096 tokens, NUM_HEADS=4, dh=32.
Sharding: 8 cores = (batch b, seq-quarter qi). Each core computes the full
attention output for 1024 query tokens of one batch element (all 4 heads),
so there are no collectives: the final channel-mixing projection is local.

Math restructuring (host-side, exact):
  - LN affine (w, b) folded into the QKV in-projection weights/bias.
  - K-bias dropped entirely (softmax is invariant to per-query shifts).
  - V-bias folded into the output-projection bias (attn rows sum to 1).
  - Projection bias folded into the residual tiles (per-channel column).
  - x pre-cast to bf16 host-side (residual kept f32 via a small side DMA).
  - Key order is permuted per-core so each core's own query quarter is
    always blocks 6-7 (softmax/PV are permutation-invariant over keys),
    letting one 8-block pass produce K, V, Q and the residual.
Device pipeline per core:
  P1 (software-pipelined stages A/B/C across 512-token blocks):
      A: DMA x block, LN mean via 1/128-ones matmul, mean PSUM->SBUF (ACT)
      B: center + square (DVE bf16), var matmul, inv-std via exp(-0.5*ln)
      C: yn = d*inv; K projection -> ksb [feat, tok] (ACT copy-out);
         V projected directly token-major (yn_chunk^T @ Wv, no transposes);
         blocks 6-7 also produce Q (+bias) and the residual.
  P2: per (query-block 512, head-pair): scores for 64 (chunk, head) units
      in granules of 3 -> one exp[128,1536] ACT call per granule; two
      row-tiled concurrent score matmuls per chunk (K=64 at PE rows 0-63 /
      64-127); col-tiled cross-paired PV/denominator matmuls accumulate
      into shared [128,512] PSUM banks one granule behind the exp stream;
      normalize with one Ln + one Exp + one multiply per pair; projections
      deferred to overlap the next pair's score stream.
"""

import os
import sys
from contextlib import ExitStack

import numpy as np

sys.path.insert(0, "/opt/trn_rl_repo")

B, C, S, SQ = 2, 128, 4096, 1024
NH, DH = 4, 32
EPS = 1e-5
SCALE = 1.0 / np.sqrt(np.float32(DH))
NB = S // 512    # 8 blocks of 512 tokens
NT = S // 128    # 32 key/value token chunks
LAST_RESULTS = None


def build_program():
    import concourse.bass as bass
    import concourse.mybir as mybir
    import concourse.tile as tile

    f32 = mybir.dt.float32
    bf16 = mybir.dt.bfloat16
    AF = mybir.ActivationFunctionType
    OP = mybir.AluOpType

    def split_multi_waits(nc):
        """walrus on this image encodes at most ONE sync wait per
        instruction; split extras into same-engine NOPs placed before."""
        def fix_block(blk):
            new_insts = []
            for inst in blk.instructions:
                try:
                    subs = inst.blocks
                except AttributeError:
                    subs = None
                if subs:
                    for sub in subs:
                        fix_block(sub)
                si = inst.sync_info
                waits = list(si.on_wait) if si is not None and si.on_wait else []
                if len(waits) > 1:
                    for j, w in enumerate(waits[:-1]):
                        nop = mybir.InstNoOp(name=f"{inst.name}-ws{j}")
                        nop.engine = inst.engine
                        nop.sync_info = mybir.SyncInfo(on_wait=[w],
                                                       on_update=[])
                        new_insts.append(nop)
                    inst.sync_info = mybir.SyncInfo(
                        on_wait=[waits[-1]], on_update=list(si.on_update))
                new_insts.append(inst)
            blk.instructions = new_insts
        for blk in nc.m.functions[0].blocks:
            fix_block(blk)

    nc = bass.Bass()

    xr_t = nc.declare_dram_parameter("xr", [C, S], bf16, isOutput=False)
    xi_t = nc.declare_dram_parameter("xi", [C, S], bf16, isOutput=False)
    # f32 residual source: own quarter only
    rqr_t = nc.declare_dram_parameter("rqr", [C, SQ], f32, isOutput=False)
    rqi_t = nc.declare_dram_parameter("rqi", [C, SQ], f32, isOutput=False)
    # K/Q projection weights: [k_p0 | k_p1 | q_p0 | q_p1], 128 cols each
    wa_t = nc.declare_dram_parameter("wa", [C, 512], bf16, isOutput=False)
    wb_t = nc.declare_dram_parameter("wb", [C, 512], bf16, isOutput=False)
    # V projection (token-major output): [p0h0|p0h1|p1h0|p1h1], 64 cols each
    wva_t = nc.declare_dram_parameter("wva", [C, 256], bf16, isOutput=False)
    wvb_t = nc.declare_dram_parameter("wvb", [C, 256], bf16, isOutput=False)
    qb2_t = nc.declare_dram_parameter("qb2", [C, 2], f32, isOutput=False)
    lp_t = nc.declare_dram_parameter("lp", [C, 512], bf16, isOutput=False)
    pbf_t = nc.declare_dram_parameter("pbf", [C, 2], f32, isOutput=False)
    out_t = nc.declare_dram_parameter("out", [2, C, SQ], f32, isOutput=True)

    with tile.TileContext(nc) as tc, ExitStack() as ctx:
        const_pool = ctx.enter_context(tc.tile_pool(name="const", bufs=1))
        big_pool = ctx.enter_context(tc.tile_pool(name="big", bufs=1))

        # First two x blocks DMA'd ahead of the weights so P1 compute can
        # start as early as possible.
        x_pre = [const_pool.tile([128, 512], bf16, tag=f"xpre{i}",
                                 name=f"xpre{i}") for i in range(4)]
        for blk in range(2):
            sl = slice(blk * 512, (blk + 1) * 512)
            nc.sync.dma_start(out=x_pre[2 * blk][:], in_=xr_t[:, sl])
            nc.sync.dma_start(out=x_pre[2 * blk + 1][:], in_=xi_t[:, sl])

        ones_bc = const_pool.tile([128, 128], bf16, tag="ones_bc", name="ones_bc")
        nc.gpsimd.memset(ones_bc[:], 1.0 / 128.0)
        ones_pv = const_pool.tile([128, 64], bf16, tag="ones_pv", name="ones_pv")
        nc.gpsimd.memset(ones_pv[:], 1.0)
        eps_c = const_pool.tile([128, 1], f32, tag="eps_c", name="eps_c")
        nc.gpsimd.memset(eps_c[:], EPS)
        # Pre-sync ACT with gpsimd consts (and trigger the exp/ln table
        # load early) so later activations carry a single sync wait.
        act_warm = const_pool.tile([128, 1], f32, tag="act_warm",
                                   name="act_warm")
        nc.scalar.activation(act_warm[:], eps_c[:], AF.Exp)

        wa = const_pool.tile([C, 512], bf16, tag="wa", name="wa")
        wb = const_pool.tile([C, 512], bf16, tag="wb", name="wb")
        wva = const_pool.tile([C, 256], bf16, tag="wva", name="wva")
        wvb = const_pool.tile([C, 256], bf16, tag="wvb", name="wvb")
        qb2 = const_pool.tile([C, 2], f32, tag="qb2", name="qb2")
        lp = const_pool.tile([C, 512], bf16, tag="lp", name="lp")
        pbf = const_pool.tile([C, 2], f32, tag="pbf", name="pbf")
        nc.sync.dma_start(out=wa[:], in_=wa_t[:])
        nc.sync.dma_start(out=wb[:], in_=wb_t[:])
        nc.sync.dma_start(out=wva[:], in_=wva_t[:])
        nc.sync.dma_start(out=wvb[:], in_=wvb_t[:])
        nc.sync.dma_start(out=qb2[:], in_=qb2_t[:])
        nc.sync.dma_start(out=lp[:], in_=lp_t[:])
        nc.sync.dma_start(out=pbf[:], in_=pbf_t[:])
        # Pre-sync DVE with the first x DMA lane so the first DVE ops carry
        # a single sync wait (walrus wait-slot limit).
        dve_warm = const_pool.tile([128, 1], bf16, tag="dve_warm",
                                   name="dve_warm")
        nc.vector.tensor_copy(dve_warm[:], x_pre[0][:, 0:1])

        # persistent activation storage
        ksb = [big_pool.tile([128, S], bf16, tag=f"ksb{p}", name=f"ksb{p}")
               for p in range(2)]
        qsb = [big_pool.tile([128, SQ], bf16, tag=f"qsb{p}", name=f"qsb{p}")
               for p in range(2)]
        # vTe: token-major V, per chunk 256 cols [p0h0|p0h1|p1h0|p1h1]
        vTe = big_pool.tile([128, NT * 256], bf16, tag="vTe", name="vTe")
        resid = [big_pool.tile([128, 512], f32, tag=f"res{i}", name=f"res{i}")
                 for i in range(4)]
        # resid order: [r blk0, r blk1, i blk0, i blk1]

        # ---------------- P1: LN + QKV projection ----------------
        with ExitStack() as p1:
            xin_pool = p1.enter_context(tc.tile_pool(name="xin", bufs=6))
            tmp_pool = p1.enter_context(tc.tile_pool(name="tmp", bufs=2))
            mu_pool = p1.enter_context(
                tc.tile_pool(name="mups", bufs=1, space="PSUM"))
            var_pool = p1.enter_context(
                tc.tile_pool(name="varps", bufs=2, space="PSUM"))
            kq_pool = p1.enter_context(
                tc.tile_pool(name="kqps", bufs=2, space="PSUM"))
            vt_pool = p1.enter_context(
                tc.tile_pool(name="vtps", bufs=2, space="PSUM"))

            st_ = [dict() for _ in range(NB)]

            def stage_a(b):
                s = st_[b]
                sl = slice(b * 512, (b + 1) * 512)
                own = b >= 6
                qb_i = b - 6
                if b < 2:
                    xb_r, xb_i = x_pre[2 * b], x_pre[2 * b + 1]
                else:
                    xb_r = xin_pool.tile([128, 512], bf16, tag="xb_r",
                                         name="xb_r")
                    xb_i = xin_pool.tile([128, 512], bf16, tag="xb_i",
                                         name="xb_i")
                    nc.sync.dma_start(out=xb_r[:], in_=xr_t[:, sl])
                    nc.sync.dma_start(out=xb_i[:], in_=xi_t[:, sl])
                if own:
                    qsl = slice(qb_i * 512, (qb_i + 1) * 512)
                    rr, ri = resid[qb_i], resid[2 + qb_i]
                    nc.sync.dma_start(out=rr[:], in_=rqr_t[:, qsl])
                    nc.sync.dma_start(out=ri[:], in_=rqi_t[:, qsl])
                    # fold projection bias into the residual (per channel)
                    nc.vector.tensor_scalar_add(rr[:], rr[:], pbf[:, 0:1])
                    nc.vector.tensor_scalar_add(ri[:], ri[:], pbf[:, 1:2])
                mu = mu_pool.tile([128, 1024], f32, tag="mu", name="mu")
                nc.tensor.matmul(mu[:, 0:512], ones_bc[:], xb_r[:],
                                 start=True, stop=True)
                nc.tensor.matmul(mu[:, 512:1024], ones_bc[:], xb_i[:],
                                 start=True, stop=True)
                mu_sb = tmp_pool.tile([128, 1024], bf16, tag="mu_sb",
                                      name="mu_sb")
                nc.scalar.activation(mu_sb[:], mu[:], AF.Copy)
                s["xb"] = (xb_r, xb_i)
                s["mu_sb"] = mu_sb

            def stage_b(b):
                s = st_[b]
                xb_r, xb_i = s["xb"]
                mu_sb = s["mu_sb"]
                d_r = tmp_pool.tile([128, 512], bf16, tag="d_r", name="d_r")
                d_i = tmp_pool.tile([128, 512], bf16, tag="d_i", name="d_i")
                nc.vector.tensor_tensor(d_r[:], xb_r[:], mu_sb[:, 0:512],
                                        OP.subtract)
                nc.vector.tensor_tensor(d_i[:], xb_i[:], mu_sb[:, 512:1024],
                                        OP.subtract)
                sq_r = tmp_pool.tile([128, 512], bf16, tag="sq_r", name="sq_r")
                sq_i = tmp_pool.tile([128, 512], bf16, tag="sq_i", name="sq_i")
                nc.vector.tensor_tensor(sq_r[:], d_r[:], d_r[:], OP.mult)
                nc.vector.tensor_tensor(sq_i[:], d_i[:], d_i[:], OP.mult)
                var = var_pool.tile([128, 512], f32, tag="var", name="var")
                nc.tensor.matmul(var[:], ones_bc[:], sq_r[:],
                                 start=True, stop=False)
                nc.tensor.matmul(var[:], ones_bc[:], sq_i[:],
                                 start=False, stop=True)
                lntmp = tmp_pool.tile([128, 512], f32, tag="lntmp",
                                      name="lntmp")
                nc.scalar.activation(lntmp[:], var[:], AF.Ln, bias=eps_c[:])
                inv = tmp_pool.tile([128, 512], bf16, tag="inv", name="inv")
                nc.scalar.activation(inv[:], lntmp[:], AF.Exp, scale=-0.5)
                s["d"] = (d_r, d_i)
                s["inv"] = inv

            def stage_c(b):
                s = st_[b]
                sl = slice(b * 512, (b + 1) * 512)
                own = b >= 6
                qb_i = b - 6
                d_r, d_i = s["d"]
                inv = s["inv"]
                yn_r = tmp_pool.tile([128, 512], bf16, tag="yn_r", name="yn_r")
                yn_i = tmp_pool.tile([128, 512], bf16, tag="yn_i", name="yn_i")
                nc.vector.tensor_tensor(yn_r[:], d_r[:], inv[:], OP.mult)
                nc.vector.tensor_tensor(yn_i[:], d_i[:], inv[:], OP.mult)

                # K tiles (both pairs), PSUM -> ksb via ACT copy (no bias)
                for p in range(2):
                    ps = kq_pool.tile([128, 512], f32, tag="kq", name="kq")
                    nc.tensor.matmul(ps[:], wa[:, p * 128:(p + 1) * 128],
                                     yn_r[:], start=True, stop=False)
                    nc.tensor.matmul(ps[:], wb[:, p * 128:(p + 1) * 128],
                                     yn_i[:], start=False, stop=True)
                    nc.scalar.activation(ksb[p][:, sl], ps[:], AF.Copy)

                # V token-major: per 128-token chunk, yn_chunk^T @ Wv;
                # two chunks share one PSUM tile -> one copy-out each
                for cc2 in range(2):
                    vt = vt_pool.tile([128, 512], f32, tag="vt", name="vt")
                    for sub in range(2):
                        cch = cc2 * 2 + sub
                        csl = slice(cch * 128, (cch + 1) * 128)
                        # start=True only on the very first matmul: its
                        # bank-wide has_written clear must precede all
                        # writes into this shared PSUM bank
                        nc.tensor.matmul(vt[:, sub * 256:(sub + 1) * 256],
                                         yn_r[:, csl], wva[:],
                                         start=(sub == 0), stop=False,
                                         skip_group_check=True)
                        nc.tensor.matmul(vt[:, sub * 256:(sub + 1) * 256],
                                         yn_i[:, csl], wvb[:],
                                         start=False, stop=(sub == 1),
                                         skip_group_check=True)
                    tt = b * 4 + cc2 * 2
                    nc.vector.tensor_copy(
                        vTe[:, tt * 256:(tt + 2) * 256], vt[:])

                if own:
                    qsl = slice(qb_i * 512, (qb_i + 1) * 512)
                    for p in range(2):
                        ps = kq_pool.tile([128, 512], f32, tag="kq",
                                          name="kq")
                        nc.tensor.matmul(ps[:],
                                         wa[:, 256 + p * 128:384 + p * 128],
                                         yn_r[:], start=True, stop=False)
                        nc.tensor.matmul(ps[:],
                                         wb[:, 256 + p * 128:384 + p * 128],
                                         yn_i[:], start=False, stop=True)
                        nc.vector.tensor_scalar_add(qsb[p][:, qsl], ps[:],
                                                    qb2[:, p:p + 1])

            for t in range(NB + 2):
                if t < NB:
                    stage_a(t)
                if 1 <= t <= NB:
                    stage_b(t - 1)
                if t >= 2:
                    stage_c(t - 2)

        # ---------------- P2: attention + projection ----------------
        with ExitStack() as p3:
            sc_pool = p3.enter_context(
                tc.tile_pool(name="scps", bufs=2, space="PSUM"))
            pv_pool = p3.enter_context(
                tc.tile_pool(name="pvps", bufs=1, space="PSUM"))
            exp_pool = p3.enter_context(tc.tile_pool(name="exp", bufs=3))
            sm_pool = p3.enter_context(tc.tile_pool(name="sm", bufs=2))
            out_pool = p3.enter_context(tc.tile_pool(name="outp", bufs=4))

            NU = 2 * NT  # (chunk, head) score units per (qblock, pair)
            grans = [list(range(g * 3, min(g * 3 + 3, NU)))
                     for g in range((NU + 2) // 3)]
            attn_t = {}

            def attn_pair(blk, pair):
                bsl = slice(blk * 512, (blk + 1) * 512)
                pv = pv_pool.tile([128, 512], f32, tag="pv", name="pv")
                den = pv_pool.tile([128, 512], f32, tag="den", name="den")
                ex_of = {}
                next_chunk = 0
                for gi in range(len(grans) + 1):
                    if gi < len(grans):
                        units = grans[gi]
                        w = 512 * len(units)
                        sc = sc_pool.tile([128, 1536], f32, tag="sc",
                                          name="sc")
                        for j, u in enumerate(units):
                            tt, h = u // 2, u % 2
                            tsl = slice(tt * 128, (tt + 1) * 128)
                            co = j * 512
                            if h == 0:
                                nc.tensor.matmul(sc[:, co:co + 512],
                                                 ksb[pair][0:64, tsl],
                                                 qsb[pair][0:64, bsl],
                                                 start=True, stop=True)
                            else:
                                nc.tensor.matmul(sc[:, co:co + 512],
                                                 ksb[pair][64:128, tsl],
                                                 qsb[pair][64:128, bsl],
                                                 start=True, stop=True,
                                                 tile_position=(64, 0))
                        ex = exp_pool.tile([128, 1536], bf16, tag="ex",
                                           name="ex")
                        nc.scalar.activation(ex[:, 0:w], sc[:, 0:w], AF.Exp,
                                             scale=float(SCALE))
                        for j, u in enumerate(units):
                            ex_of[u] = (ex, j * 512)
                    ready = grans[gi - 1][-1] if gi >= 1 else -1
                    while (next_chunk < NT
                           and 2 * next_chunk + 1 <= ready):
                        tt = next_chunk
                        ex0, c0 = ex_of.pop(2 * tt)
                        ex1, c1 = ex_of.pop(2 * tt + 1)
                        st, sp = tt == 0, tt == NT - 1
                        vb = tt * 256 + pair * 128
                        # col-tiled cross pairs: (pv_h0 | den_h1) and
                        # (pv_h1 | den_h0) run concurrently on the PE
                        nc.tensor.matmul(pv[0:64, :],
                                         vTe[:, vb:vb + 64],
                                         ex0[:, c0:c0 + 512],
                                         start=st, stop=sp,
                                         skip_group_check=True)
                        nc.tensor.matmul(den[64:128, :], ones_pv[:],
                                         ex1[:, c1:c1 + 512],
                                         start=st, stop=sp,
                                         tile_position=(0, 64),
                                         skip_group_check=True)
                        nc.tensor.matmul(pv[64:128, :],
                                         vTe[:, vb + 64:vb + 128],
                                         ex1[:, c1:c1 + 512],
                                         start=False, stop=sp,
                                         tile_position=(0, 64),
                                         skip_group_check=True)
                        nc.tensor.matmul(den[0:64, :], ones_pv[:],
                                         ex0[:, c0:c0 + 512],
                                         start=False, stop=sp,
                                         skip_group_check=True)
                        next_chunk += 1
                rtmp = sm_pool.tile([128, 512], f32, tag="rtmp", name="rtmp")
                nc.scalar.activation(rtmp[:], den[:], AF.Ln)
                r_bc = sm_pool.tile([128, 512], f32, tag="r_bc", name="r_bc")
                nc.scalar.activation(r_bc[:], rtmp[:], AF.Exp, scale=-1.0)
                at = sm_pool.tile([128, 512], bf16, tag=f"attn{pair}",
                                  name=f"attn{pair}")
                nc.vector.tensor_tensor(at[:], pv[:], r_bc[:], OP.mult)
                attn_t[(blk, pair)] = at

            def proj(blk):
                bsl = slice(blk * 512, (blk + 1) * 512)
                for comp in range(2):  # 0=real, 1=imag
                    ps = pv_pool.tile([128, 512], f32,
                                      tag=("pv" if comp == 0 else "den"),
                                      name="proj")
                    nc.tensor.matmul(ps[:], lp[:, comp * 128:(comp + 1) * 128],
                                     attn_t[(blk, 0)][:],
                                     start=True, stop=False)
                    nc.tensor.matmul(ps[:],
                                     lp[:, 256 + comp * 128:384 + comp * 128],
                                     attn_t[(blk, 1)][:],
                                     start=False, stop=True)
                    # halved residual-add + DMA so the first half's store
                    # starts while the second half is still adding
                    o_sb = out_pool.tile([128, 512], f32, tag="o_sb",
                                         name="o_sb")
                    for h in range(2):
                        hs = slice(h * 256, (h + 1) * 256)
                        os_ = slice(blk * 512 + h * 256,
                                    blk * 512 + (h + 1) * 256)
                        nc.vector.tensor_tensor(
                            o_sb[:, hs], ps[:, hs],
                            resid[2 * comp + blk][:, hs], OP.add)
                        nc.sync.dma_start(out=out_t[comp, :, os_],
                                          in_=o_sb[:, hs])

            attn_pair(0, 0)
            attn_pair(0, 1)
            attn_pair(1, 0)
            proj(0)
            attn_pair(1, 1)
            proj(1)
    split_multi_waits(nc)
    return nc


def pack_inputs(inputs):
    """Host-side exact restructuring; returns per-core input maps."""
    f = lambda k: np.asarray(inputs[k], np.float32)
    xr = f("x_real").reshape(B, C, S)
    xi = f("x_imag").reshape(B, C, S)
    Win = (f("in_w_r") + 1j * f("in_w_i")).astype(np.complex64)
    lnw = (f("ln_w_r") + 1j * f("ln_w_i")).astype(np.complex64)
    lnb = (f("ln_b_r") + 1j * f("ln_b_i")).astype(np.complex64)
    inb = (f("in_b_r") + 1j * f("in_b_i")).astype(np.complex64)
    Wp = Win * lnw[None, :]
    biasq = inb + Win @ lnb
    Wout = (f("out_w_r") + 1j * f("out_w_i")).astype(np.complex64)
    Wc = (f("conv_w_r") + 1j * f("conv_w_i")).astype(np.complex64)
    outb = (f("out_b_r") + 1j * f("out_b_i")).astype(np.complex64)
    convb = (f("conv_b_r") + 1j * f("conv_b_i")).astype(np.complex64)
    M = Wc @ Wout
    vb_ = biasq[2 * C:3 * C]
    bM = Wc @ outb + convb + M @ vb_  # v-bias folded (attn rows sum to 1)

    def pack_pair(Wsec, h0):
        W0 = Wsec[32 * h0:32 * h0 + 32]
        W1 = Wsec[32 * (h0 + 1):32 * (h0 + 1) + 32]
        RA = np.concatenate([W0.real, W0.imag, W1.real, W1.imag], 0)
        RB = np.concatenate([-W0.imag, W0.real, -W1.imag, W1.real], 0)
        return RA.T.copy(), RB.T.copy()

    qW, kW, vW = Wp[0:C], Wp[C:2 * C], Wp[2 * C:3 * C]
    qb_ = biasq[0:C]
    tiles = [pack_pair(kW, 0), pack_pair(kW, 2),
             pack_pair(qW, 0), pack_pair(qW, 2)]
    import ml_dtypes
    bf = ml_dtypes.bfloat16
    wa = np.ascontiguousarray(
        np.concatenate([t[0] for t in tiles], 1)).astype(bf)
    wb = np.ascontiguousarray(
        np.concatenate([t[1] for t in tiles], 1)).astype(bf)
    vt = [pack_pair(vW, 0), pack_pair(vW, 2)]
    wva = np.ascontiguousarray(
        np.concatenate([t[0] for t in vt], 1)).astype(bf)
    wvb = np.ascontiguousarray(
        np.concatenate([t[1] for t in vt], 1)).astype(bf)

    def pack_qbias(h0):
        b0 = qb_[32 * h0:32 * h0 + 32]
        b1 = qb_[32 * (h0 + 1):32 * (h0 + 1) + 32]
        return np.concatenate([b0.real, b0.imag, b1.real, b1.imag], 0)
    qb2 = np.ascontiguousarray(
        np.stack([pack_qbias(0), pack_qbias(2)], 1), np.float32)

    def pack_proj(h0):
        M0 = M[:, 32 * h0:32 * h0 + 32]
        M1 = M[:, 32 * (h0 + 1):32 * (h0 + 1) + 32]
        Lr = np.concatenate([M0.real.T, -M0.imag.T, M1.real.T, -M1.imag.T], 0)
        Li = np.concatenate([M0.imag.T, M0.real.T, M1.imag.T, M1.real.T], 0)
        return Lr, Li
    L01r, L01i = pack_proj(0)
    L23r, L23i = pack_proj(2)
    lp = np.ascontiguousarray(
        np.concatenate([L01r, L01i, L23r, L23i], 1)).astype(bf)
    pbf = np.ascontiguousarray(
        np.stack([bM.real, bM.imag], 1), np.float32)

    xr16 = xr.astype(bf)
    xi16 = xi.astype(bf)
    in_maps = []
    for core in range(8):
        b, qi = core // 4, core % 4
        # permute key order: own quarter last (blocks 6-7)
        order = [q for q in range(4) if q != qi] + [qi]
        xrp = np.concatenate(
            [xr16[b][:, q * SQ:(q + 1) * SQ] for q in order], 1)
        xip = np.concatenate(
            [xi16[b][:, q * SQ:(q + 1) * SQ] for q in order], 1)
        qsl = slice(qi * SQ, (qi + 1) * SQ)
        in_maps.append({
            "xr": np.ascontiguousarray(xrp),
            "xi": np.ascontiguousarray(xip),
            "rqr": np.ascontiguousarray(xr[b][:, qsl]),
            "rqi": np.ascontiguousarray(xi[b][:, qsl]),
            "wa": wa, "wb": wb, "wva": wva, "wvb": wvb,
            "qb2": qb2, "lp": lp, "pbf": pbf,
        })
    return in_maps


_CACHED = {}


def _ensure_ntff_hook():
    """Register the axon NTFF profiling hook (absent from this image's
    antenv) so run_bass_kernel_spmd(trace=True) can capture HW timing."""
    try:
        import antenv.axon_hooks  # noqa: F401
        return
    except ImportError:
        pass
    import types

    try:
        from trn_agent_boot import trn_boot
        hook = trn_boot._ntff_profile_via_ctypes("/opt/axon/libaxon_pjrt.so")
    except Exception:
        return
    import antenv

    mod = types.ModuleType("antenv.axon_hooks")
    mod.get_axon_ntff_profile_hook = lambda: hook
    mod.set_axon_ntff_profile_hook = lambda h: None
    sys.modules["antenv.axon_hooks"] = mod
    antenv.axon_hooks = mod


def kernel(trace=False, **inputs):
    global LAST_RESULTS
    from concourse.bass_utils import run_bass_kernel_spmd

    if trace:
        _ensure_ntff_hook()

    if "nc" not in _CACHED:
        _CACHED["nc"] = build_program()
    nc = _CACHED["nc"]
    in_maps = pack_inputs(inputs)
    res = run_bass_kernel_spmd(nc, in_maps, core_ids=list(range(8)),
                               trace=trace)
    LAST_RESULTS = res
    out = np.zeros((2, B, C, S), np.float32)
    for core in range(8):
        b, qi = core // 4, core % 4
        out[:, b, :, qi * SQ:(qi + 1) * SQ] = res.results[core]["out"]
    return out.reshape(2, B, C, 64, 64)
